# revision 39
# baseline (speedup 1.0000x reference)
"""MoE transformer block on 8 TRN2 NeuronCores.

Launch A (data-parallel over tokens): per core = (batch b, seq half) -> 512
query tokens.  Attention path in fp32r (router-accuracy critical), shared MLP
in bf16.  Outputs partial = x2 + shared, h2 (bf16), router logits (fp32).

Host: top-2 routing, gates, aux loss, per-expert token dispatch.

Launch B (expert-parallel): core e runs expert e's MLP over its C dispatched
tokens, bf16 weights/activations.

Host: gated combine -> full output.
"""

import math
import numpy as np
import ml_dtypes
from contextlib import ExitStack

import concourse.bass as bass
import concourse.tile as tile
from concourse import bacc, mybir
from concourse.bass_utils import run_bass_kernel_spmd

F32 = mybir.dt.float32
F32R = mybir.dt.float32r
BF16 = mybir.dt.bfloat16
AF = mybir.ActivationFunctionType

B, S, E, H, NE, TOPK = 4, 1024, 1024, 16, 8, 2
DFF = 4 * E
DH = E // H
EPS = 1e-5
NCORES = 8
TOK = 512          # own tokens per core in launch A
NK = E // 128      # 8 feature tiles

_cache = {}
last_results = []   # [(name, BassKernelResults), ...] for test harness
last_programs = []  # [(name, Bacc), ...] for test harness timing


def _build_launch_a(trace=False):
    nc = bacc.Bacc("TRN2", target_bir_lowering=False, debug=False,
                   enable_asserts=True, num_devices=NCORES)
    d = {}
    def di(name, shape, dt):
        d[name] = nc.dram_tensor(name, shape, dt, kind="ExternalInput").ap()
    def do(name, shape, dt):
        d[name] = nc.dram_tensor(name, shape, dt, kind="ExternalOutput").ap()

    di("xb", [E, S], F32R)            # x[batch].T, own half first
    for w in ("wq", "wk", "wv", "wo"):
        di(w, [E, E], F32R)
    for b in ("bq", "bk", "bo"):
        di(b, [E, 1], F32)
    di("bvb", [128, E], F32)          # bv broadcast across partitions
    di("ln1_g", [E, 1], F32); di("ln1_b", [E, 1], F32)
    di("ln2_g", [E, 1], F32); di("ln2_b", [E, 1], F32)
    di("sw1", [E, DFF], BF16); di("sb1", [DFF, 1], F32)
    di("sw2", [DFF, E], BF16); di("sb2", [E, 1], F32)
    di("wg", [E, NE], F32)
    di("onesd", [128, 128], F32R)
    do("partial", [E, TOK], F32)      # x + attn + shared  (FM)
    do("h2out", [E, TOK], BF16)
    do("logits", [NE, TOK], F32)

    with tile.TileContext(nc) as tc, ExitStack() as ctx:
        pacc = ctx.enter_context(tc.tile_pool(name="pacc", bufs=8, space="PSUM"))
        const = ctx.enter_context(tc.tile_pool(name="const", bufs=1))
        wpool = ctx.enter_context(tc.tile_pool(name="wpool", bufs=3))
        stat = ctx.enter_context(tc.tile_pool(name="stat", bufs=2))
        tmp = ctx.enter_context(tc.tile_pool(name="tmp", bufs=2))

        ones = const.tile([128, 128], F32R, tag="ones")
        nc.sync.dma_start(ones[:], d["onesd"][:])
        bvb_sb = const.tile([128, E], F32, tag="bvb")
        nc.sync.dma_start(bvb_sb[:], d["bvb"][:])

        def bias_tiles(name, n, tag):
            t = const.tile([128, n], F32, tag=tag, name=f"bt_{tag}")
            nc.sync.dma_start(t[:].rearrange("p (k o) -> p k o", o=1),
                              d[name].rearrange("(k p) o -> p k o", p=128))
            return [t[:, k:k + 1] for k in range(n)]

        g1 = bias_tiles("ln1_g", NK, "g1"); b1 = bias_tiles("ln1_b", NK, "b1")
        g2 = bias_tiles("ln2_g", NK, "g2"); b2 = bias_tiles("ln2_b", NK, "b2")
        bq = bias_tiles("bq", NK, "bq"); bk = bias_tiles("bk", NK, "bk")
        bo = bias_tiles("bo", NK, "bo")
        sb1t = bias_tiles("sb1", DFF // 128, "sb1")
        sb2t = bias_tiles("sb2", NK, "sb2")

        def layernorm(src_r, src_f, gt, bt, ncols, outs, chunk_sel=None):
            """src_r(k, sl)->fp32r AP (matmul rhs / Square in);
            src_f(k, sl)->fp32 AP; outs(k, c)->dest AP [128,512]."""
            nch = ncols // 512
            chunks = chunk_sel if chunk_sel is not None else range(nch)
            for c in chunks:
                sl = slice(c * 512, c * 512 + 512)
                ms = pacc.tile([1, 512], F32, tag="mm", name="ms")
                qs = pacc.tile([1, 512], F32, tag="mm", name="qs")
                for k in range(NK):
                    sr = src_r(k, sl)
                    nc.tensor.matmul(ms[:], ones[:, 0:1], sr,
                                     start=(k == 0), stop=(k == NK - 1))
                    xsq = tmp.tile([128, 512], F32R, tag="xsq", name="xsq")
                    nc.scalar.activation(xsq[:], sr, AF.Square)
                    nc.tensor.matmul(qs[:], ones[:, 0:1], xsq[:],
                                     start=(k == 0), stop=(k == NK - 1))
                m_sb = stat.tile([1, 512], F32, tag="scr", bufs=6, name="m_sb")
                nc.scalar.activation(m_sb[:], ms[:], AF.Copy, scale=1.0 / E)
                mq_sb = stat.tile([1, 512], F32, tag="scr", bufs=6, name="mq_sb")
                nc.scalar.activation(mq_sb[:], qs[:], AF.Copy, scale=1.0 / E)
                m2 = stat.tile([1, 512], F32, tag="scr", bufs=6, name="m2")
                nc.scalar.activation(m2[:], m_sb[:], AF.Square)
                var = stat.tile([1, 512], F32, tag="scr", bufs=6, name="var")
                nc.vector.tensor_sub(var[:], mq_sb[:], m2[:])
                nc.vector.tensor_scalar_add(var[:], var[:], EPS)
                std = stat.tile([1, 512], F32, tag="scr", bufs=6, name="std")
                nc.scalar.activation(std[:], var[:], AF.Sqrt)
                rstd = stat.tile([1, 512], F32, tag="scr", bufs=6, name="rstd")
                nc.vector.reciprocal(rstd[:], std[:])
                m_r = stat.tile([1, 512], F32R, tag="scr", bufs=6, name="m_r")
                nc.vector.tensor_copy(m_r[:], m_sb[:])
                r_r = stat.tile([1, 512], F32R, tag="scr", bufs=6, name="r_r")
                nc.vector.tensor_copy(r_r[:], rstd[:])
                mb = pacc.tile([128, 512], F32, tag="mm", name="mb")
                nc.tensor.matmul(mb[:], ones[0:1, :], m_r[:], start=True, stop=True)
                rb = pacc.tile([128, 512], F32, tag="mm", name="rb")
                nc.tensor.matmul(rb[:], ones[0:1, :], r_r[:], start=True, stop=True)
                for k in range(NK):
                    t1 = tmp.tile([128, 512], F32, tag="t1", bufs=3, name="t1")
                    nc.vector.tensor_sub(t1[:], src_f(k, sl), mb[:])
                    nc.vector.tensor_mul(t1[:], t1[:], rb[:])
                    nc.vector.tensor_scalar(outs(k, c), t1[:], gt[k][:],
                                            bt[k][:], mybir.AluOpType.mult,
                                            mybir.AluOpType.add)

        cpool = ctx.enter_context(tc.tile_pool(name="cpool", bufs=1))
        if True:
            s_attn = ExitStack()
            qpool = s_attn.enter_context(tc.tile_pool(name="qpool", bufs=1))
            kpool = s_attn.enter_context(tc.tile_pool(name="kpool", bufs=1))
            vpool = s_attn.enter_context(tc.tile_pool(name="vpool", bufs=1))

            with tc.tile_pool(name="h1pool", bufs=1) as h1pool:
                h1 = [h1pool.tile([128, S], F32R, tag=f"h1{k}", name=f"h1{k}")
                      for k in range(NK)]

                with tc.tile_pool(name="xstrp", bufs=3) as xstrp:
                    def ln1_srcr(k, sl):
                        t = xstrp.tile([128, 512], F32R, tag="xstr",
                                       name="xstr")
                        nc.sync.dma_start(t[:],
                                          d["xb"][k * 128:(k + 1) * 128, sl])
                        return t[:]

                    def ln1_srcf(k, sl):
                        t = xstrp.tile([128, 512], F32R, tag="xstr",
                                       name="xstr2")
                        nc.sync.dma_start(t[:],
                                          d["xb"][k * 128:(k + 1) * 128, sl])
                        return t[:].bitcast(F32)

                    layernorm(ln1_srcr, ln1_srcf, g1, b1, S,
                              lambda k, c: h1[k][:, c * 512:(c + 1) * 512],
                              chunk_sel=[0])

                    # ---- Q (own 512 tokens) — overlaps LN1 chunk 1 ----
                    qps = [pacc.tile([128, 512], F32, tag="mm", name=f"qps{i}")
                           for i in range(NK)]
                    for k in range(NK):
                        wt = wpool.tile([128, E], F32R, tag="w", bufs=4,
                                        name="wqk")
                        nc.sync.dma_start(wt[:], d["wq"][k * 128:(k + 1) * 128, :])
                        for m in range(NK):
                            nc.tensor.matmul(qps[m][:],
                                             wt[:, m * 128:(m + 1) * 128],
                                             h1[k][:, 0:512], start=(k == 0),
                                             stop=(k == NK - 1))
                    qsb = []
                    for m in range(NK):
                        t = qpool.tile([128, 512], F32R, tag=f"q{m}",
                                       name=f"q{m}")
                        nc.vector.tensor_scalar_add(t[:], qps[m][:], bq[m][:])
                        qsb.append(t)

                    layernorm(ln1_srcr, ln1_srcf, g1, b1, S,
                              lambda k, c: h1[k][:, c * 512:(c + 1) * 512],
                              chunk_sel=[1])

                # ---- K (all 1024 tokens, two chunk passes) ----
                ksb = [kpool.tile([128, S], F32R, tag=f"k{m}", name=f"ksb{m}")
                       for m in range(NK)]
                for c2 in range(2):
                    kps = [pacc.tile([128, 512], F32, tag="mm", name=f"kps{i}")
                           for i in range(NK)]
                    for k in range(NK):
                        wt = wpool.tile([128, E], F32R, tag="w", bufs=4, name="wkk")
                        nc.sync.dma_start(wt[:], d["wk"][k * 128:(k + 1) * 128, :])
                        for m in range(NK):
                            nc.tensor.matmul(kps[m][:],
                                             wt[:, m * 128:(m + 1) * 128],
                                             h1[k][:, c2 * 512:(c2 + 1) * 512],
                                             start=(k == 0), stop=(k == NK - 1))
                    for m in range(NK):
                        nc.vector.tensor_scalar_add(
                            ksb[m][:, c2 * 512:(c2 + 1) * 512],
                            kps[m][:], bk[m][:])

                # ---- V (token-major, 65-strided per head, ones col) ----
                vsb = [vpool.tile([128, 16 * 65], F32R, tag=f"v{t}",
                                  name=f"vsb{t}") for t in range(NK)]
                for t in range(NK):
                    ov = vsb[t][:].rearrange("p (h e) -> p h e", e=65)[:, :, 64:65]
                    nc.scalar.copy(ov, ones[:, 0:16].rearrange(
                        "p (h e) -> p h e", e=1))
                for tg in range(2):
                    vps = {}
                    for t in range(4):
                        for f in range(2):
                            vps[(t, f)] = pacc.tile([128, 512], F32, tag="mm",
                                                    name=f"vps{t}_{f}")
                    for k in range(NK):
                        wt = wpool.tile([128, E], F32R, tag="w", bufs=4, name="wvk")
                        nc.sync.dma_start(wt[:], d["wv"][k * 128:(k + 1) * 128, :])
                        for t in range(4):
                            tt = tg * 4 + t
                            for f in range(2):
                                nc.tensor.matmul(
                                    vps[(t, f)][:],
                                    h1[k][:, tt * 128:tt * 128 + 128],
                                    wt[:, f * 512:(f + 1) * 512],
                                    start=(k == 0), stop=(k == NK - 1))
                    for t in range(4):
                        for f in range(2):
                            dst = vsb[tg * 4 + t][:, f * 520:(f + 1) * 520] \
                                .rearrange("p (h e) -> p h e", e=65)[:, :, 0:64]
                            src = vps[(t, f)][:].rearrange("p (h e) -> p h e", e=64)
                            nc.vector.tensor_add(
                                dst, src,
                                bvb_sb[:, f * 512:(f + 1) * 512].rearrange(
                                    "p (h e) -> p h e", e=64))
            # h1 freed here

            # ---- attention per head ----
            ppool = s_attn.enter_context(tc.tile_pool(name="ppool", bufs=6))
            stat2 = s_attn.enter_context(tc.tile_pool(name="stat2", bufs=2))
            packed = [cpool.tile([128, 512], F32R, tag=f"c{p}", name=f"packed{p}")
                      for p in range(NK)]
            LOOKAHEAD = 2
            pending_norm = []
            for hp in range(H // 2):
                heads = (2 * hp, 2 * hp + 1)
                cps = {}
                for h in heads:
                    cps[h] = pacc.tile([65, 512], F32, tag="mm", name=f"cps{h}")

                pend = {}  # kc -> {h: exp tile}

                def emit_s(kc):
                    psbs = {}
                    for h in heads:
                        ktile = ksb[hp][(h % 2) * 64:(h % 2) * 64 + 64, :]
                        qtile = qsb[hp][(h % 2) * 64:(h % 2) * 64 + 64, :]
                        sps = pacc.tile([128, 512], F32, tag="mm",
                                        name=f"sps{h}_{kc}")
                        nc.tensor.matmul(sps[:],
                                         ktile[:, kc * 128:kc * 128 + 128],
                                         qtile[:], start=True, stop=True)
                        psb = ppool.tile([128, 512], F32R, tag="p",
                                         name=f"p{h}_{kc}")
                        nc.scalar.activation(psb[:], sps[:], AF.Exp,
                                             scale=1.0 / math.sqrt(DH))
                        psbs[h] = psb
                    pend[kc] = psbs

                def emit_pv(kc):
                    psbs = pend.pop(kc)
                    for h in heads:
                        nc.tensor.matmul(cps[h][:],
                                         vsb[kc][:, h * 65:h * 65 + 65],
                                         psbs[h][:], start=(kc == 0),
                                         stop=(kc == NK - 1))

                for kc in range(NK):
                    emit_s(kc)
                    if kc >= LOOKAHEAD:
                        emit_pv(kc - LOOKAHEAD)
                for kc in range(NK - LOOKAHEAD, NK):
                    emit_pv(kc)

                def normalize(hp=hp, cps=cps, heads=heads):
                    for h in heads:
                        rd = stat2.tile([1, 512], F32, tag="rd", name="rd")
                        nc.vector.reciprocal(rd[:], cps[h][64:65, :])
                        rdr = stat2.tile([1, 512], F32R, tag="rdr", name="rdr")
                        nc.vector.tensor_copy(rdr[:], rd[:])
                        bcp = pacc.tile([64, 512], F32, tag="mm", name=f"bcp{h}")
                        nc.tensor.matmul(bcp[:], ones[0:1, 0:64], rdr[:],
                                         start=True, stop=True)
                        bcs = tmp.tile([64, 512], F32, tag="sf", bufs=4,
                                       name="bcs")
                        nc.vector.tensor_copy(bcs[:], bcp[:])
                        if h % 2 == 0:
                            nc.vector.tensor_mul(packed[hp][0:64, :],
                                                 cps[h][0:64, :], bcs[:])
                        else:
                            ct = tmp.tile([64, 512], F32R, tag="sf", bufs=4,
                                          name="ct")
                            nc.vector.tensor_mul(ct[:], cps[h][0:64, :], bcs[:])
                            nc.sync.dma_start(packed[hp][64:128, :], ct[:])

                pending_norm.append(normalize)
                if len(pending_norm) > 1:
                    pending_norm.pop(0)()
            while pending_norm:
                pending_norm.pop(0)()
            s_attn.close()  # q/k/v/p freed

            # ---- O-proj + residual ----
            x2pool = ctx.enter_context(tc.tile_pool(name="x2pool", bufs=1))
            ops = [pacc.tile([128, 512], F32, tag="mm", name=f"ops{i}")
                   for i in range(NK)]
            for k in range(NK):
                wt = wpool.tile([128, E], F32R, tag="w", bufs=4, name="wok")
                nc.sync.dma_start(wt[:], d["wo"][k * 128:(k + 1) * 128, :])
                for m in range(NK):
                    nc.tensor.matmul(ops[m][:], wt[:, m * 128:(m + 1) * 128],
                                     packed[k][:], start=(k == 0),
                                     stop=(k == NK - 1))
            x2 = []
            x2r = []
            for m in range(NK):
                xot = tmp.tile([128, 512], F32, tag="sf", bufs=4, name="xot")
                nc.sync.dma_start(xot[:], d["xb"][m * 128:(m + 1) * 128, 0:512]
                                  .bitcast(F32))
                osb = tmp.tile([128, 512], F32, tag="sf", bufs=4, name="osb")
                nc.vector.tensor_scalar_add(osb[:], ops[m][:], bo[m][:])
                t = x2pool.tile([128, 512], F32, tag=f"x2{m}", name=f"x2_{m}")
                nc.vector.tensor_add(t[:], osb[:], xot[:])
                x2.append(t)
                tr = x2pool.tile([128, 512], F32R, tag=f"x2r{m}", name=f"x2r{m}")
                nc.vector.tensor_copy(tr[:], t[:])
                x2r.append(tr)
        # xown, packed freed

        # ---- LN2 ----
        h2pool = ctx.enter_context(tc.tile_pool(name="h2pool", bufs=1))
        outp = ctx.enter_context(tc.tile_pool(name="outp", bufs=2))
        h2f = [h2pool.tile([128, 512], F32, tag=f"h2f{k}", name=f"h2f{k}")
               for k in range(NK)]
        layernorm(lambda k, sl: x2r[k][:, sl], lambda k, sl: x2[k][:, sl],
                  g2, b2, 512, lambda k, c: h2f[k][:])
        h2b = []
        for k in range(NK):
            t = h2pool.tile([128, 512], BF16, tag=f"h2b{k}", name=f"h2b{k}")
            nc.vector.tensor_copy(t[:], h2f[k][:])
            h2b.append(t)
            nc.sync.dma_start(d["h2out"][k * 128:(k + 1) * 128, :], t[:])

        # ---- router logits (full fp32) ----
        wgt = const.tile([128, NE * NK], F32, tag="wg")
        nc.sync.dma_start(wgt[:].rearrange("p (k e) -> p k e", e=NE),
                          d["wg"].rearrange("(k p) e -> p k e", p=128))
        gps = pacc.tile([NE, 512], F32, tag="mm", name="gps")
        for k in range(NK):
            nc.tensor.matmul(gps[:], wgt[:, k * NE:(k + 1) * NE], h2f[k][:],
                             start=(k == 0), stop=(k == NK - 1))
        lsb = outp.tile([NE, 512], F32, tag="l", name="lsb")
        nc.vector.tensor_copy(lsb[:], gps[:])
        nc.sync.dma_start(d["logits"][:], lsb[:])

        # ---- shared MLP (bf16) ----
        with tc.tile_pool(name="midpool", bufs=1) as midpool:
            mid = []
            for jg in range(4):
                mps = [pacc.tile([128, 512], F32, tag="mm", name=f"mps{jg}_{i}")
                       for i in range(8)]
                for k in range(NK):
                    wt = wpool.tile([128, 1024], BF16, tag="wb", bufs=8, name="sw1k")
                    nc.sync.dma_start(wt[:], d["sw1"][k * 128:(k + 1) * 128,
                                                      jg * 1024:(jg + 1) * 1024])
                    for j in range(8):
                        nc.tensor.matmul(mps[j][:], wt[:, j * 128:(j + 1) * 128],
                                         h2b[k][:], start=(k == 0),
                                         stop=(k == NK - 1))
                for j in range(8):
                    jj = jg * 8 + j
                    t = midpool.tile([128, 512], BF16, tag=f"mid{jj}",
                                     name=f"mid{jj}")
                    nc.scalar.activation(t[:], mps[j][:], AF.Gelu,
                                         bias=sb1t[jj][:])
                    mid.append(t)
            for mg in range(2):
                o2 = [pacc.tile([128, 512], F32, tag="mm", name=f"o2_{i}")
                      for i in range(4)]
                for j in range(DFF // 128):
                    wt = wpool.tile([128, 512], BF16, tag="wb", bufs=8,
                                    name="sw2j")
                    nc.sync.dma_start(wt[:], d["sw2"][j * 128:(j + 1) * 128,
                                                      mg * 512:(mg + 1) * 512])
                    for m in range(4):
                        nc.tensor.matmul(o2[m][:], wt[:, m * 128:(m + 1) * 128],
                                         mid[j][:], start=(j == 0),
                                         stop=(j == DFF // 128 - 1))
                for m in range(4):
                    mm2 = mg * 4 + m
                    sh = tmp.tile([128, 512], F32, tag="sf", bufs=4, name="sh")
                    nc.vector.tensor_scalar_add(sh[:], o2[m][:], sb2t[mm2][:])
                    pt = outp.tile([128, 512], F32, tag="pt", name="pt")
                    nc.vector.tensor_add(pt[:], sh[:], x2[mm2][:])
                    nc.sync.dma_start(d["partial"][mm2 * 128:(mm2 + 1) * 128, :],
                                      pt[:])

    nc.compile()
    return nc


def _build_launch_b(chunks):
    nc = bacc.Bacc("TRN2", target_bir_lowering=False, debug=False,
                   enable_asserts=True, num_devices=NCORES)
    C = sum(chunks)
    d = {}
    d["h2d"] = nc.dram_tensor("h2d", [E, C], BF16, kind="ExternalInput").ap()
    d["e1"] = nc.dram_tensor("e1", [E, DFF], BF16, kind="ExternalInput").ap()
    d["e2"] = nc.dram_tensor("e2", [DFF, E], BF16, kind="ExternalInput").ap()
    d["b1"] = nc.dram_tensor("b1", [DFF, 1], F32, kind="ExternalInput").ap()
    d["b2"] = nc.dram_tensor("b2", [E, 1], F32, kind="ExternalInput").ap()
    d["yout"] = nc.dram_tensor("yout", [E, C], F32, kind="ExternalOutput").ap()

    with tile.TileContext(nc) as tc, ExitStack() as ctx:
        pacc = ctx.enter_context(tc.tile_pool(name="pacc", bufs=8, space="PSUM"))
        const = ctx.enter_context(tc.tile_pool(name="const", bufs=1))
        hpool = ctx.enter_context(tc.tile_pool(name="hpool", bufs=1))
        wpool = ctx.enter_context(tc.tile_pool(name="wpool", bufs=14))
        midpool = ctx.enter_context(tc.tile_pool(name="midpool", bufs=1))
        ypool = ctx.enter_context(tc.tile_pool(name="ypool", bufs=2))

        b1w = const.tile([128, DFF // 128], F32, tag="b1w", name="b1w")
        nc.sync.dma_start(b1w[:].rearrange("p (k o) -> p k o", o=1),
                          d["b1"].rearrange("(k p) o -> p k o", p=128))
        b1t = [b1w[:, k:k + 1] for k in range(DFF // 128)]
        b2w = const.tile([128, NK], F32, tag="b2w", name="b2w")
        nc.sync.dma_start(b2w[:].rearrange("p (k o) -> p k o", o=1),
                          d["b2"].rearrange("(k p) o -> p k o", p=128))
        b2t = [b2w[:, k:k + 1] for k in range(NK)]
        h2d = []
        for k in range(NK):
            t = hpool.tile([128, C], BF16, tag=f"h{k}", name=f"h2d{k}")
            nc.sync.dma_start(t[:], d["h2d"][k * 128:(k + 1) * 128, :])
            h2d.append(t)

        off = 0
        for ci, cw in enumerate(chunks):
            csl = slice(off, off + cw)
            mid = []
            for jg in range(4):
                mps = [pacc.tile([128, cw], F32, tag="mm", name=f"bmps{i}")
                       for i in range(8)]
                for k in range(NK):
                    wt = wpool.tile([128, 1024], BF16, tag="w1", name="wt")
                    nc.sync.dma_start(wt[:], d["e1"][k * 128:(k + 1) * 128,
                                                     jg * 1024:(jg + 1) * 1024])
                    for j in range(8):
                        nc.tensor.matmul(mps[j][:], wt[:, j * 128:(j + 1) * 128],
                                         h2d[k][:, csl], start=(k == 0),
                                         stop=(k == NK - 1))
                for j in range(8):
                    jj = jg * 8 + j
                    t = midpool.tile([128, cw], BF16, tag=f"mid{jj}_{ci % 2}",
                                     name=f"bmid{jj}")
                    nc.scalar.activation(t[:], mps[j][:], AF.Gelu,
                                         bias=b1t[jj][:])
                    mid.append(t)
            o2 = [pacc.tile([128, cw], F32, tag="mm", name=f"bo2_{i}")
                  for i in range(NK)]
            for j in range(DFF // 128):
                wt = wpool.tile([128, 1024], BF16, tag="w2", name="wt2")
                nc.sync.dma_start(wt[:], d["e2"][j * 128:(j + 1) * 128, :])
                for m in range(NK):
                    nc.tensor.matmul(o2[m][:], wt[:, m * 128:(m + 1) * 128],
                                     mid[j][:], start=(j == 0),
                                     stop=(j == DFF // 128 - 1))
            for m in range(NK):
                y = ypool.tile([128, cw], F32, tag="y", name="y")
                nc.scalar.activation(y[:], o2[m][:], AF.Identity,
                                     bias=b2t[m][:])
                nc.sync.dma_start(d["yout"][m * 128:(m + 1) * 128, csl], y[:])
            off += cw

    nc.compile()
    return nc


def _chunk_sizes(C):
    n = (C + 511) // 512
    base = C // n // 128 * 128
    sizes = [base] * n
    rem = C - base * n
    i = 0
    while rem > 0:
        sizes[i] += 128
        rem -= 128
        i = (i + 1) % n
    assert sum(sizes) == C and all(s <= 512 for s in sizes)
    return sizes


def kernel(**inputs):
    global last_results, last_programs
    last_results = []
    last_programs = []

    f32 = lambda a: np.ascontiguousarray(np.asarray(a), dtype=np.float32)
    x = f32(inputs["x"])
    col = lambda a: f32(a).reshape(-1, 1)

    if "A" not in _cache:
        _cache["A"] = _build_launch_a()
    ncA = _cache["A"]

    wq, wk, wv, wo = (f32(inputs[k]) for k in ("wq", "wk", "wv", "wo"))
    sw1 = f32(inputs["sw1"]).astype(ml_dtypes.bfloat16)
    sw2 = f32(inputs["sw2"]).astype(ml_dtypes.bfloat16)
    onesd = np.ones((128, 128), np.float32)
    bvb = np.broadcast_to(f32(inputs["bv"]), (128, E)).copy()
    shared_in = dict(
        wq=wq, wk=wk, wv=wv, wo=wo,
        bq=col(inputs["bq"]), bk=col(inputs["bk"]), bo=col(inputs["bo"]),
        bvb=bvb,
        ln1_g=col(inputs["ln1_g"]), ln1_b=col(inputs["ln1_b"]),
        ln2_g=col(inputs["ln2_g"]), ln2_b=col(inputs["ln2_b"]),
        sw1=sw1, sb1=col(inputs["sb1"]), sw2=sw2, sb2=col(inputs["sb2"]),
        wg=f32(inputs["w_gate"]), onesd=onesd,
    )
    in_maps = []
    for c in range(NCORES):
        b, half = c // 2, c % 2
        xt = x[b].T  # [E, S]
        own = xt[:, half * 512:(half + 1) * 512]
        oth = xt[:, (1 - half) * 512:(2 - half) * 512]
        xb = np.ascontiguousarray(np.concatenate([own, oth], axis=1))
        in_maps.append({**shared_in, "xb": xb})

    resA = run_bass_kernel_spmd(ncA, in_maps, core_ids=list(range(NCORES)))
    last_results.append(("A", resA))
    last_programs.append(("A", ncA))

    partial = np.concatenate([resA.results[c]["partial"].T for c in range(NCORES)], 0)
    h2bf = np.concatenate([resA.results[c]["h2out"] for c in range(NCORES)], 1)
    logits = np.concatenate([resA.results[c]["logits"].T for c in range(NCORES)], 0)

    # ---- routing on host (mirrors reference, fp32) ----
    N = B * S
    order = np.argsort(-logits, axis=-1, kind="stable")
    top_idx = order[:, :TOPK]
    top_vals = np.take_along_axis(logits, top_idx, axis=-1)
    tv = top_vals - top_vals.max(-1, keepdims=True)
    te = np.exp(tv, dtype=np.float32)
    top_gates = te / te.sum(-1, keepdims=True)
    gates_dense = np.zeros((N, NE), np.float32)
    np.put_along_axis(gates_dense, top_idx, top_gates, axis=-1)
    lm = logits - logits.max(-1, keepdims=True)
    le = np.exp(lm, dtype=np.float32)
    probs = le / le.sum(-1, keepdims=True)
    P = probs.mean(0, dtype=np.float32)
    f = (gates_dense > 0).astype(np.float32).mean(0, dtype=np.float32)
    aux = np.float32(NE * np.sum(P * f, dtype=np.float32))

    # ---- dispatch ----
    sel_lists = [np.nonzero((top_idx == e).any(-1))[0] for e in range(NE)]
    counts = np.array([len(t) for t in sel_lists])
    C = int(max(512, -(-counts.max() // 128) * 128))
    chunks = tuple(_chunk_sizes(C))
    key = ("B", chunks)
    if key not in _cache:
        _cache[key] = _build_launch_b(list(chunks))
    ncB = _cache[key]

    in_maps_b = []
    ew1 = np.asarray(inputs["ew1"]).astype(ml_dtypes.bfloat16)
    ew2 = np.asarray(inputs["ew2"]).astype(ml_dtypes.bfloat16)
    eb1 = f32(inputs["eb1"]); eb2 = f32(inputs["eb2"])
    idxs = []
    for e in range(NE):
        idx = np.zeros(C, np.int64)
        idx[:counts[e]] = sel_lists[e]
        idxs.append(idx)
        h2d = np.ascontiguousarray(h2bf[:, idx])
        in_maps_b.append(dict(h2d=h2d, e1=np.ascontiguousarray(ew1[e]),
                              e2=np.ascontiguousarray(ew2[e]),
                              b1=eb1[e].reshape(-1, 1).astype(np.float32),
                              b2=eb2[e].reshape(-1, 1).astype(np.float32)))
    resB = run_bass_kernel_spmd(ncB, in_maps_b, core_ids=list(range(NCORES)))
    last_results.append(("B", resB))
    last_programs.append(("B", ncB))

    out = partial
    for e in range(NE):
        cnt = counts[e]
        if cnt == 0:
            continue
        y = resB.results[e]["yout"][:, :cnt].T  # [cnt, E]
        g = gates_dense[idxs[e][:cnt], e][:, None]
        out[idxs[e][:cnt]] += g * y
    return out.reshape(B, S, E).astype(np.float32), aux


# revision 40
# speedup vs baseline: 1.0281x; 1.0281x over previous
"""MoE transformer block on 8 TRN2 NeuronCores.

Launch A (data-parallel over tokens): per core = (batch b, seq half) -> 512
query tokens.  Attention path in fp32r (router-accuracy critical), shared MLP
in bf16.  Outputs partial = x2 + shared, h2 (bf16), router logits (fp32).

Host: top-2 routing, gates, aux loss, per-expert token dispatch.

Launch B (expert-parallel): core e runs expert e's MLP over its C dispatched
tokens, bf16 weights/activations.

Host: gated combine -> full output.
"""

import math
import numpy as np
import ml_dtypes
from contextlib import ExitStack

import concourse.bass as bass
import concourse.tile as tile
from concourse import bacc, mybir
from concourse.bass_utils import run_bass_kernel_spmd

F32 = mybir.dt.float32
F32R = mybir.dt.float32r
BF16 = mybir.dt.bfloat16
AF = mybir.ActivationFunctionType

B, S, E, H, NE, TOPK = 4, 1024, 1024, 16, 8, 2
DFF = 4 * E
DH = E // H
EPS = 1e-5
NCORES = 8
TOK = 512          # own tokens per core in launch A
NK = E // 128      # 8 feature tiles

_cache = {}
last_results = []   # [(name, BassKernelResults), ...] for test harness
last_programs = []  # [(name, Bacc), ...] for test harness timing


def _build_launch_a(trace=False):
    nc = bacc.Bacc("TRN2", target_bir_lowering=False, debug=False,
                   enable_asserts=True, num_devices=NCORES)
    d = {}
    def di(name, shape, dt):
        d[name] = nc.dram_tensor(name, shape, dt, kind="ExternalInput").ap()
    def do(name, shape, dt):
        d[name] = nc.dram_tensor(name, shape, dt, kind="ExternalOutput").ap()

    di("xb", [E, S], F32R)            # x[batch].T, own half first
    for w in ("wq", "wk", "wv", "wo"):
        di(w, [E, E], F32R)
    for b in ("bq", "bk", "bo"):
        di(b, [E, 1], F32)
    di("bvb", [128, E], F32)          # bv broadcast across partitions
    di("ln1_g", [E, 1], F32); di("ln1_b", [E, 1], F32)
    di("ln2_g", [E, 1], F32); di("ln2_b", [E, 1], F32)
    di("sw1", [E, DFF], BF16); di("sb1", [DFF, 1], F32)
    di("sw2", [DFF, E], BF16); di("sb2", [E, 1], F32)
    di("wg", [E, NE], F32)
    di("onesd", [128, 128], F32R)
    do("partial", [E, TOK], F32)      # x + attn + shared  (FM)
    do("h2out", [E, TOK], BF16)
    do("logits", [NE, TOK], F32)

    with tile.TileContext(nc) as tc, ExitStack() as ctx:
        pacc = ctx.enter_context(tc.tile_pool(name="pacc", bufs=8, space="PSUM"))
        const = ctx.enter_context(tc.tile_pool(name="const", bufs=1))
        wpool = ctx.enter_context(tc.tile_pool(name="wpool", bufs=3))
        stat = ctx.enter_context(tc.tile_pool(name="stat", bufs=2))
        tmp = ctx.enter_context(tc.tile_pool(name="tmp", bufs=2))

        ones = const.tile([128, 128], F32R, tag="ones")
        nc.sync.dma_start(ones[:], d["onesd"][:])
        bvb_sb = const.tile([128, E], F32, tag="bvb")
        nc.sync.dma_start(bvb_sb[:], d["bvb"][:])

        def bias_tiles(name, n, tag):
            t = const.tile([128, n], F32, tag=tag, name=f"bt_{tag}")
            nc.sync.dma_start(t[:].rearrange("p (k o) -> p k o", o=1),
                              d[name].rearrange("(k p) o -> p k o", p=128))
            return [t[:, k:k + 1] for k in range(n)]

        g1 = bias_tiles("ln1_g", NK, "g1"); b1 = bias_tiles("ln1_b", NK, "b1")
        g2 = bias_tiles("ln2_g", NK, "g2"); b2 = bias_tiles("ln2_b", NK, "b2")
        bq = bias_tiles("bq", NK, "bq"); bk = bias_tiles("bk", NK, "bk")
        bo = bias_tiles("bo", NK, "bo")
        sb1t = bias_tiles("sb1", DFF // 128, "sb1")
        sb2t = bias_tiles("sb2", NK, "sb2")

        def layernorm(src_r, src_f, gt, bt, ncols, outs, chunk_sel=None):
            """src_r(k, sl)->fp32r AP (matmul rhs / Square in);
            src_f(k, sl)->fp32 AP; outs(k, c)->dest AP [128,512]."""
            nch = ncols // 512
            chunks = chunk_sel if chunk_sel is not None else range(nch)
            for c in chunks:
                sl = slice(c * 512, c * 512 + 512)
                ms = pacc.tile([1, 512], F32, tag="mm", name="ms")
                qs = pacc.tile([1, 512], F32, tag="mm", name="qs")
                for k in range(NK):
                    sr = src_r(k, sl)
                    nc.tensor.matmul(ms[:], ones[:, 0:1], sr,
                                     start=(k == 0), stop=(k == NK - 1))
                    xsq = tmp.tile([128, 512], F32R, tag="xsq", name="xsq")
                    nc.scalar.activation(xsq[:], sr, AF.Square)
                    nc.tensor.matmul(qs[:], ones[:, 0:1], xsq[:],
                                     start=(k == 0), stop=(k == NK - 1))
                m_sb = stat.tile([1, 512], F32, tag="scr", bufs=6, name="m_sb")
                nc.scalar.activation(m_sb[:], ms[:], AF.Copy, scale=1.0 / E)
                mq_sb = stat.tile([1, 512], F32, tag="scr", bufs=6, name="mq_sb")
                nc.scalar.activation(mq_sb[:], qs[:], AF.Copy, scale=1.0 / E)
                m2 = stat.tile([1, 512], F32, tag="scr", bufs=6, name="m2")
                nc.scalar.activation(m2[:], m_sb[:], AF.Square)
                var = stat.tile([1, 512], F32, tag="scr", bufs=6, name="var")
                nc.vector.tensor_sub(var[:], mq_sb[:], m2[:])
                nc.vector.tensor_scalar_add(var[:], var[:], EPS)
                std = stat.tile([1, 512], F32, tag="scr", bufs=6, name="std")
                nc.scalar.activation(std[:], var[:], AF.Sqrt)
                rstd = stat.tile([1, 512], F32, tag="scr", bufs=6, name="rstd")
                nc.vector.reciprocal(rstd[:], std[:])
                m_r = stat.tile([1, 512], F32R, tag="scr", bufs=6, name="m_r")
                nc.vector.tensor_copy(m_r[:], m_sb[:])
                r_r = stat.tile([1, 512], F32R, tag="scr", bufs=6, name="r_r")
                nc.vector.tensor_copy(r_r[:], rstd[:])
                mb = pacc.tile([128, 512], F32, tag="mm", name="mb")
                nc.tensor.matmul(mb[:], ones[0:1, :], m_r[:], start=True, stop=True)
                rb = pacc.tile([128, 512], F32, tag="mm", name="rb")
                nc.tensor.matmul(rb[:], ones[0:1, :], r_r[:], start=True, stop=True)
                for k in range(NK):
                    t1 = tmp.tile([128, 512], F32, tag="t1", bufs=3, name="t1")
                    nc.vector.tensor_sub(t1[:], src_f(k, sl), mb[:])
                    nc.vector.tensor_mul(t1[:], t1[:], rb[:])
                    nc.vector.tensor_scalar(outs(k, c), t1[:], gt[k][:],
                                            bt[k][:], mybir.AluOpType.mult,
                                            mybir.AluOpType.add)

        cpool = ctx.enter_context(tc.tile_pool(name="cpool", bufs=1))
        if True:
            s_attn = ExitStack()
            qpool = s_attn.enter_context(tc.tile_pool(name="qpool", bufs=1))
            kpool = s_attn.enter_context(tc.tile_pool(name="kpool", bufs=1))
            vpool = s_attn.enter_context(tc.tile_pool(name="vpool", bufs=1))

            with tc.tile_pool(name="h1pool", bufs=1) as h1pool:
                h1 = [h1pool.tile([128, S], F32R, tag=f"h1{k}", name=f"h1{k}")
                      for k in range(NK)]

                with tc.tile_pool(name="xstrp", bufs=3) as xstrp:
                    def ln1_srcr(k, sl):
                        t = xstrp.tile([128, 512], F32R, tag="xstr",
                                       name="xstr")
                        nc.sync.dma_start(t[:],
                                          d["xb"][k * 128:(k + 1) * 128, sl])
                        return t[:]

                    def ln1_srcf(k, sl):
                        t = xstrp.tile([128, 512], F32R, tag="xstr",
                                       name="xstr2")
                        nc.sync.dma_start(t[:],
                                          d["xb"][k * 128:(k + 1) * 128, sl])
                        return t[:].bitcast(F32)

                    layernorm(ln1_srcr, ln1_srcf, g1, b1, S,
                              lambda k, c: h1[k][:, c * 512:(c + 1) * 512],
                              chunk_sel=[0])

                    # ---- Q (own 512 tokens) — overlaps LN1 chunk 1 ----
                    qps = [pacc.tile([128, 512], F32, tag="mm", name=f"qps{i}")
                           for i in range(NK)]
                    for k in range(NK):
                        wt = wpool.tile([128, E], F32R, tag="w", bufs=4,
                                        name="wqk")
                        nc.sync.dma_start(wt[:], d["wq"][k * 128:(k + 1) * 128, :])
                        for m in range(NK):
                            nc.tensor.matmul(qps[m][:],
                                             wt[:, m * 128:(m + 1) * 128],
                                             h1[k][:, 0:512], start=(k == 0),
                                             stop=(k == NK - 1))
                    qsb = []
                    for m in range(NK):
                        t = qpool.tile([128, 512], F32R, tag=f"q{m}",
                                       name=f"q{m}")
                        nc.vector.tensor_scalar_add(t[:], qps[m][:], bq[m][:])
                        qsb.append(t)

                    layernorm(ln1_srcr, ln1_srcf, g1, b1, S,
                              lambda k, c: h1[k][:, c * 512:(c + 1) * 512],
                              chunk_sel=[1])

                # ---- K (all 1024 tokens, two chunk passes) ----
                ksb = [kpool.tile([128, S], F32R, tag=f"k{m}", name=f"ksb{m}")
                       for m in range(NK)]
                for c2 in range(2):
                    kps = [pacc.tile([128, 512], F32, tag="mm", name=f"kps{i}")
                           for i in range(NK)]
                    for k in range(NK):
                        wt = wpool.tile([128, E], F32R, tag="w", bufs=4, name="wkk")
                        nc.sync.dma_start(wt[:], d["wk"][k * 128:(k + 1) * 128, :])
                        for m in range(NK):
                            nc.tensor.matmul(kps[m][:],
                                             wt[:, m * 128:(m + 1) * 128],
                                             h1[k][:, c2 * 512:(c2 + 1) * 512],
                                             start=(k == 0), stop=(k == NK - 1))
                    for m in range(NK):
                        nc.vector.tensor_scalar_add(
                            ksb[m][:, c2 * 512:(c2 + 1) * 512],
                            kps[m][:], bk[m][:])

                # ---- V (token-major, 65-strided per head, ones col) ----
                vsb = [vpool.tile([128, 16 * 65], F32R, tag=f"v{t}",
                                  name=f"vsb{t}") for t in range(NK)]
                for t in range(NK):
                    ov = vsb[t][:].rearrange("p (h e) -> p h e", e=65)[:, :, 64:65]
                    nc.scalar.copy(ov, ones[:, 0:16].rearrange(
                        "p (h e) -> p h e", e=1))
                for tg in range(2):
                    vps = {}
                    for t in range(4):
                        for f in range(2):
                            vps[(t, f)] = pacc.tile([128, 512], F32, tag="mm",
                                                    name=f"vps{t}_{f}")
                    for k in range(NK):
                        wt = wpool.tile([128, E], F32R, tag="w", bufs=4, name="wvk")
                        nc.sync.dma_start(wt[:], d["wv"][k * 128:(k + 1) * 128, :])
                        for t in range(4):
                            tt = tg * 4 + t
                            for f in range(2):
                                nc.tensor.matmul(
                                    vps[(t, f)][:],
                                    h1[k][:, tt * 128:tt * 128 + 128],
                                    wt[:, f * 512:(f + 1) * 512],
                                    start=(k == 0), stop=(k == NK - 1))
                    for t in range(4):
                        for f in range(2):
                            dst = vsb[tg * 4 + t][:, f * 520:(f + 1) * 520] \
                                .rearrange("p (h e) -> p h e", e=65)[:, :, 0:64]
                            src = vps[(t, f)][:].rearrange("p (h e) -> p h e", e=64)
                            nc.vector.tensor_add(
                                dst, src,
                                bvb_sb[:, f * 512:(f + 1) * 512].rearrange(
                                    "p (h e) -> p h e", e=64))
            # h1 freed here

            # ---- attention per head ----
            ppool = s_attn.enter_context(tc.tile_pool(name="ppool", bufs=6))
            stat2 = s_attn.enter_context(tc.tile_pool(name="stat2", bufs=2))
            packed = [cpool.tile([128, 512], F32R, tag=f"c{p}", name=f"packed{p}")
                      for p in range(NK)]
            LOOKAHEAD = 2
            pending_norm = []
            for hp in range(H // 2):
                heads = (2 * hp, 2 * hp + 1)
                cps = {}
                for h in heads:
                    cps[h] = pacc.tile([65, 512], F32, tag="mm", name=f"cps{h}")

                pend = {}  # kc -> {h: exp tile}

                def emit_s(kc):
                    psbs = {}
                    for h in heads:
                        ktile = ksb[hp][(h % 2) * 64:(h % 2) * 64 + 64, :]
                        qtile = qsb[hp][(h % 2) * 64:(h % 2) * 64 + 64, :]
                        sps = pacc.tile([128, 512], F32, tag="mm",
                                        name=f"sps{h}_{kc}")
                        nc.tensor.matmul(sps[:],
                                         ktile[:, kc * 128:kc * 128 + 128],
                                         qtile[:], start=True, stop=True)
                        psb = ppool.tile([128, 512], F32R, tag="p",
                                         name=f"p{h}_{kc}")
                        nc.scalar.activation(psb[:], sps[:], AF.Exp,
                                             scale=1.0 / math.sqrt(DH))
                        psbs[h] = psb
                    pend[kc] = psbs

                def emit_pv(kc):
                    psbs = pend.pop(kc)
                    for h in heads:
                        nc.tensor.matmul(cps[h][:],
                                         vsb[kc][:, h * 65:h * 65 + 65],
                                         psbs[h][:], start=(kc == 0),
                                         stop=(kc == NK - 1))

                for kc in range(NK):
                    emit_s(kc)
                    if kc >= LOOKAHEAD:
                        emit_pv(kc - LOOKAHEAD)
                for kc in range(NK - LOOKAHEAD, NK):
                    emit_pv(kc)

                def normalize(hp=hp, cps=cps, heads=heads):
                    for h in heads:
                        rd = stat2.tile([1, 512], F32, tag="rd", name="rd")
                        nc.vector.reciprocal(rd[:], cps[h][64:65, :])
                        rdr = stat2.tile([1, 512], F32R, tag="rdr", name="rdr")
                        nc.vector.tensor_copy(rdr[:], rd[:])
                        bcp = pacc.tile([64, 512], F32, tag="mm", name=f"bcp{h}")
                        nc.tensor.matmul(bcp[:], ones[0:1, 0:64], rdr[:],
                                         start=True, stop=True)
                        bcs = tmp.tile([64, 512], F32, tag="sf", bufs=4,
                                       name="bcs")
                        nc.vector.tensor_copy(bcs[:], bcp[:])
                        if h % 2 == 0:
                            nc.vector.tensor_mul(packed[hp][0:64, :],
                                                 cps[h][0:64, :], bcs[:])
                        else:
                            ct = tmp.tile([64, 512], F32R, tag="sf", bufs=4,
                                          name="ct")
                            nc.vector.tensor_mul(ct[:], cps[h][0:64, :], bcs[:])
                            nc.sync.dma_start(packed[hp][64:128, :], ct[:])

                pending_norm.append(normalize)
                if len(pending_norm) > 1:
                    pending_norm.pop(0)()
            while pending_norm:
                pending_norm.pop(0)()
            s_attn.close()  # q/k/v/p freed

            # ---- O-proj + residual ----
            x2pool = ctx.enter_context(tc.tile_pool(name="x2pool", bufs=1))
            ops = [pacc.tile([128, 512], F32, tag="mm", name=f"ops{i}")
                   for i in range(NK)]
            for k in range(NK):
                wt = wpool.tile([128, E], F32R, tag="w", bufs=4, name="wok")
                nc.sync.dma_start(wt[:], d["wo"][k * 128:(k + 1) * 128, :])
                for m in range(NK):
                    nc.tensor.matmul(ops[m][:], wt[:, m * 128:(m + 1) * 128],
                                     packed[k][:], start=(k == 0),
                                     stop=(k == NK - 1))
            x2 = []
            x2r = []
            for m in range(NK):
                xot = tmp.tile([128, 512], F32, tag="sf", bufs=4, name="xot")
                nc.sync.dma_start(xot[:], d["xb"][m * 128:(m + 1) * 128, 0:512]
                                  .bitcast(F32))
                osb = tmp.tile([128, 512], F32, tag="sf", bufs=4, name="osb")
                nc.vector.tensor_scalar_add(osb[:], ops[m][:], bo[m][:])
                t = x2pool.tile([128, 512], F32, tag=f"x2{m}", name=f"x2_{m}")
                nc.vector.tensor_add(t[:], osb[:], xot[:])
                x2.append(t)
                tr = x2pool.tile([128, 512], F32R, tag=f"x2r{m}", name=f"x2r{m}")
                nc.vector.tensor_copy(tr[:], t[:])
                x2r.append(tr)
        # xown, packed freed

        # ---- LN2 ----
        h2pool = ctx.enter_context(tc.tile_pool(name="h2pool", bufs=1))
        outp = ctx.enter_context(tc.tile_pool(name="outp", bufs=2))
        h2f = [h2pool.tile([128, 512], F32, tag=f"h2f{k}", name=f"h2f{k}")
               for k in range(NK)]
        layernorm(lambda k, sl: x2r[k][:, sl], lambda k, sl: x2[k][:, sl],
                  g2, b2, 512, lambda k, c: h2f[k][:])
        h2b = []
        for k in range(NK):
            t = h2pool.tile([128, 512], BF16, tag=f"h2b{k}", name=f"h2b{k}")
            nc.vector.tensor_copy(t[:], h2f[k][:])
            h2b.append(t)
            nc.sync.dma_start(d["h2out"][k * 128:(k + 1) * 128, :], t[:])

        # ---- router logits (full fp32) ----
        wgt = const.tile([128, NE * NK], F32, tag="wg")
        nc.sync.dma_start(wgt[:].rearrange("p (k e) -> p k e", e=NE),
                          d["wg"].rearrange("(k p) e -> p k e", p=128))
        gps = pacc.tile([NE, 512], F32, tag="mm", name="gps")
        for k in range(NK):
            nc.tensor.matmul(gps[:], wgt[:, k * NE:(k + 1) * NE], h2f[k][:],
                             start=(k == 0), stop=(k == NK - 1))
        lsb = outp.tile([NE, 512], F32, tag="l", name="lsb")
        nc.vector.tensor_copy(lsb[:], gps[:])
        nc.sync.dma_start(d["logits"][:], lsb[:])

        # ---- shared MLP (bf16) ----
        with tc.tile_pool(name="midpool", bufs=1) as midpool:
            mid = []
            for jg in range(4):
                mps = [pacc.tile([128, 512], F32, tag="mm", name=f"mps{jg}_{i}")
                       for i in range(8)]
                for k in range(NK):
                    wt = wpool.tile([128, 1024], BF16, tag="wb", bufs=8, name="sw1k")
                    nc.sync.dma_start(wt[:], d["sw1"][k * 128:(k + 1) * 128,
                                                      jg * 1024:(jg + 1) * 1024])
                    for j in range(8):
                        nc.tensor.matmul(mps[j][:], wt[:, j * 128:(j + 1) * 128],
                                         h2b[k][:], start=(k == 0),
                                         stop=(k == NK - 1))
                for j in range(8):
                    jj = jg * 8 + j
                    t = midpool.tile([128, 512], BF16, tag=f"mid{jj}",
                                     name=f"mid{jj}")
                    nc.scalar.activation(t[:], mps[j][:], AF.Gelu,
                                         bias=sb1t[jj][:])
                    mid.append(t)
            for mg in range(2):
                o2 = [pacc.tile([128, 512], F32, tag="mm", name=f"o2_{i}")
                      for i in range(4)]
                for j in range(DFF // 128):
                    wt = wpool.tile([128, 512], BF16, tag="wb", bufs=8,
                                    name="sw2j")
                    nc.sync.dma_start(wt[:], d["sw2"][j * 128:(j + 1) * 128,
                                                      mg * 512:(mg + 1) * 512])
                    for m in range(4):
                        nc.tensor.matmul(o2[m][:], wt[:, m * 128:(m + 1) * 128],
                                         mid[j][:], start=(j == 0),
                                         stop=(j == DFF // 128 - 1))
                for m in range(4):
                    mm2 = mg * 4 + m
                    sh = tmp.tile([128, 512], F32, tag="sf", bufs=4, name="sh")
                    nc.vector.tensor_scalar_add(sh[:], o2[m][:], sb2t[mm2][:])
                    pt = outp.tile([128, 512], F32, tag="pt", name="pt")
                    nc.vector.tensor_add(pt[:], sh[:], x2[mm2][:])
                    nc.sync.dma_start(d["partial"][mm2 * 128:(mm2 + 1) * 128, :],
                                      pt[:])

    nc.compile()
    return nc


def _build_launch_b(chunks):
    nc = bacc.Bacc("TRN2", target_bir_lowering=False, debug=False,
                   enable_asserts=True, num_devices=NCORES)
    C = sum(chunks)
    d = {}
    d["h2d"] = nc.dram_tensor("h2d", [E, C], BF16, kind="ExternalInput").ap()
    d["e1"] = nc.dram_tensor("e1", [E, DFF], BF16, kind="ExternalInput").ap()
    d["e2"] = nc.dram_tensor("e2", [DFF, E], BF16, kind="ExternalInput").ap()
    d["b1"] = nc.dram_tensor("b1", [DFF, 1], F32, kind="ExternalInput").ap()
    d["b2"] = nc.dram_tensor("b2", [E, 1], F32, kind="ExternalInput").ap()
    d["yout"] = nc.dram_tensor("yout", [E, C], F32, kind="ExternalOutput").ap()

    with tile.TileContext(nc) as tc, ExitStack() as ctx:
        pacc = ctx.enter_context(tc.tile_pool(name="pacc", bufs=8, space="PSUM"))
        const = ctx.enter_context(tc.tile_pool(name="const", bufs=1))
        hpool = ctx.enter_context(tc.tile_pool(name="hpool", bufs=1))
        wpool = ctx.enter_context(tc.tile_pool(name="wpool", bufs=14))
        midpool = ctx.enter_context(tc.tile_pool(name="midpool", bufs=1))
        ypool = ctx.enter_context(tc.tile_pool(name="ypool", bufs=2))

        b1w = const.tile([128, DFF // 128], F32, tag="b1w", name="b1w")
        nc.sync.dma_start(b1w[:].rearrange("p (k o) -> p k o", o=1),
                          d["b1"].rearrange("(k p) o -> p k o", p=128))
        b1t = [b1w[:, k:k + 1] for k in range(DFF // 128)]
        b2w = const.tile([128, NK], F32, tag="b2w", name="b2w")
        nc.sync.dma_start(b2w[:].rearrange("p (k o) -> p k o", o=1),
                          d["b2"].rearrange("(k p) o -> p k o", p=128))
        b2t = [b2w[:, k:k + 1] for k in range(NK)]
        h2d = []
        for k in range(NK):
            t = hpool.tile([128, C], BF16, tag=f"h{k}", name=f"h2d{k}")
            nc.sync.dma_start(t[:], d["h2d"][k * 128:(k + 1) * 128, :])
            h2d.append(t)

        off = 0
        for ci, cw in enumerate(chunks):
            csl = slice(off, off + cw)
            mid = []
            for jg in range(4):
                mps = [pacc.tile([128, cw], F32, tag="mm", name=f"bmps{i}")
                       for i in range(8)]
                for k in range(NK):
                    wt = wpool.tile([128, 1024], BF16, tag="w1", name="wt")
                    nc.sync.dma_start(wt[:], d["e1"][k * 128:(k + 1) * 128,
                                                     jg * 1024:(jg + 1) * 1024])
                    for j in range(8):
                        nc.tensor.matmul(mps[j][:], wt[:, j * 128:(j + 1) * 128],
                                         h2d[k][:, csl], start=(k == 0),
                                         stop=(k == NK - 1))
                for j in range(8):
                    jj = jg * 8 + j
                    t = midpool.tile([128, cw], BF16, tag=f"mid{jj}_{ci % 2}",
                                     name=f"bmid{jj}")
                    nc.scalar.activation(t[:], mps[j][:], AF.Gelu,
                                         bias=b1t[jj][:])
                    mid.append(t)
            o2 = [pacc.tile([128, cw], F32, tag="mm", name=f"bo2_{i}")
                  for i in range(NK)]
            for j in range(DFF // 128):
                wt = wpool.tile([128, 1024], BF16, tag="w2", name="wt2")
                nc.sync.dma_start(wt[:], d["e2"][j * 128:(j + 1) * 128, :])
                for m in range(NK):
                    nc.tensor.matmul(o2[m][:], wt[:, m * 128:(m + 1) * 128],
                                     mid[j][:], start=(j == 0),
                                     stop=(j == DFF // 128 - 1))
            for m in range(NK):
                y = ypool.tile([128, cw], F32, tag="y", name="y")
                nc.scalar.activation(y[:], o2[m][:], AF.Identity,
                                     bias=b2t[m][:])
                nc.sync.dma_start(d["yout"][m * 128:(m + 1) * 128, csl], y[:])
            off += cw

    nc.compile()
    return nc


def _chunk_sizes(C):
    n = (C + 511) // 512
    base = C // n // 8 * 8
    sizes = [base] * n
    rem = C - base * n
    i = 0
    while rem > 0:
        step = min(8, rem)
        sizes[i] += step
        rem -= step
        i = (i + 1) % n
    assert sum(sizes) == C and all(s <= 512 for s in sizes)
    return sizes


def kernel(**inputs):
    global last_results, last_programs
    last_results = []
    last_programs = []

    f32 = lambda a: np.ascontiguousarray(np.asarray(a), dtype=np.float32)
    x = f32(inputs["x"])
    col = lambda a: f32(a).reshape(-1, 1)

    if "A" not in _cache:
        _cache["A"] = _build_launch_a()
    ncA = _cache["A"]

    wq, wk, wv, wo = (f32(inputs[k]) for k in ("wq", "wk", "wv", "wo"))
    sw1 = f32(inputs["sw1"]).astype(ml_dtypes.bfloat16)
    sw2 = f32(inputs["sw2"]).astype(ml_dtypes.bfloat16)
    onesd = np.ones((128, 128), np.float32)
    bvb = np.broadcast_to(f32(inputs["bv"]), (128, E)).copy()
    shared_in = dict(
        wq=wq, wk=wk, wv=wv, wo=wo,
        bq=col(inputs["bq"]), bk=col(inputs["bk"]), bo=col(inputs["bo"]),
        bvb=bvb,
        ln1_g=col(inputs["ln1_g"]), ln1_b=col(inputs["ln1_b"]),
        ln2_g=col(inputs["ln2_g"]), ln2_b=col(inputs["ln2_b"]),
        sw1=sw1, sb1=col(inputs["sb1"]), sw2=sw2, sb2=col(inputs["sb2"]),
        wg=f32(inputs["w_gate"]), onesd=onesd,
    )
    in_maps = []
    for c in range(NCORES):
        b, half = c // 2, c % 2
        xt = x[b].T  # [E, S]
        own = xt[:, half * 512:(half + 1) * 512]
        oth = xt[:, (1 - half) * 512:(2 - half) * 512]
        xb = np.ascontiguousarray(np.concatenate([own, oth], axis=1))
        in_maps.append({**shared_in, "xb": xb})

    resA = run_bass_kernel_spmd(ncA, in_maps, core_ids=list(range(NCORES)))
    last_results.append(("A", resA))
    last_programs.append(("A", ncA))

    partial = np.concatenate([resA.results[c]["partial"].T for c in range(NCORES)], 0)
    h2bf = np.concatenate([resA.results[c]["h2out"] for c in range(NCORES)], 1)
    logits = np.concatenate([resA.results[c]["logits"].T for c in range(NCORES)], 0)

    # ---- routing on host (mirrors reference, fp32) ----
    N = B * S
    order = np.argsort(-logits, axis=-1, kind="stable")
    top_idx = order[:, :TOPK]
    top_vals = np.take_along_axis(logits, top_idx, axis=-1)
    tv = top_vals - top_vals.max(-1, keepdims=True)
    te = np.exp(tv, dtype=np.float32)
    top_gates = te / te.sum(-1, keepdims=True)
    gates_dense = np.zeros((N, NE), np.float32)
    np.put_along_axis(gates_dense, top_idx, top_gates, axis=-1)
    lm = logits - logits.max(-1, keepdims=True)
    le = np.exp(lm, dtype=np.float32)
    probs = le / le.sum(-1, keepdims=True)
    P = probs.mean(0, dtype=np.float32)
    f = (gates_dense > 0).astype(np.float32).mean(0, dtype=np.float32)
    aux = np.float32(NE * np.sum(P * f, dtype=np.float32))

    # ---- dispatch ----
    sel_lists = [np.nonzero((top_idx == e).any(-1))[0] for e in range(NE)]
    counts = np.array([len(t) for t in sel_lists])
    C = int(max(512, -(-counts.max() // 8) * 8))
    chunks = tuple(_chunk_sizes(C))
    key = ("B", chunks)
    if key not in _cache:
        _cache[key] = _build_launch_b(list(chunks))
    ncB = _cache[key]

    in_maps_b = []
    ew1 = np.asarray(inputs["ew1"]).astype(ml_dtypes.bfloat16)
    ew2 = np.asarray(inputs["ew2"]).astype(ml_dtypes.bfloat16)
    eb1 = f32(inputs["eb1"]); eb2 = f32(inputs["eb2"])
    idxs = []
    for e in range(NE):
        idx = np.zeros(C, np.int64)
        idx[:counts[e]] = sel_lists[e]
        idxs.append(idx)
        h2d = np.ascontiguousarray(h2bf[:, idx])
        in_maps_b.append(dict(h2d=h2d, e1=np.ascontiguousarray(ew1[e]),
                              e2=np.ascontiguousarray(ew2[e]),
                              b1=eb1[e].reshape(-1, 1).astype(np.float32),
                              b2=eb2[e].reshape(-1, 1).astype(np.float32)))
    resB = run_bass_kernel_spmd(ncB, in_maps_b, core_ids=list(range(NCORES)))
    last_results.append(("B", resB))
    last_programs.append(("B", ncB))

    out = partial
    for e in range(NE):
        cnt = counts[e]
        if cnt == 0:
            continue
        y = resB.results[e]["yout"][:, :cnt].T  # [cnt, E]
        g = gates_dense[idxs[e][:cnt], e][:, None]
        out[idxs[e][:cnt]] += g * y
    return out.reshape(B, S, E).astype(np.float32), aux


# revision 41
# speedup vs baseline: 1.0502x; 1.0215x over previous
"""MoE transformer block on 8 TRN2 NeuronCores.

Launch A (data-parallel over tokens): per core = (batch b, seq half) -> 512
query tokens.  Attention path in fp32r (router-accuracy critical), shared MLP
in bf16.  Outputs partial = x2 + shared, h2 (bf16), router logits (fp32).

Host: top-2 routing, gates, aux loss, per-expert token dispatch.

Launch B (expert-parallel): core e runs expert e's MLP over its C dispatched
tokens, bf16 weights/activations.

Host: gated combine -> full output.
"""

import math
import numpy as np
import ml_dtypes
from contextlib import ExitStack

import concourse.bass as bass
import concourse.tile as tile
from concourse import bacc, mybir
from concourse.bass_utils import run_bass_kernel_spmd

F32 = mybir.dt.float32
F32R = mybir.dt.float32r
BF16 = mybir.dt.bfloat16
AF = mybir.ActivationFunctionType

B, S, E, H, NE, TOPK = 4, 1024, 1024, 16, 8, 2
DFF = 4 * E
DH = E // H
EPS = 1e-5
NCORES = 8
TOK = 512          # own tokens per core in launch A
NK = E // 128      # 8 feature tiles

_cache = {}
last_results = []   # [(name, BassKernelResults), ...] for test harness
last_programs = []  # [(name, Bacc), ...] for test harness timing


def _build_launch_a(trace=False):
    nc = bacc.Bacc("TRN2", target_bir_lowering=False, debug=False,
                   enable_asserts=True, num_devices=NCORES)
    d = {}
    def di(name, shape, dt):
        d[name] = nc.dram_tensor(name, shape, dt, kind="ExternalInput").ap()
    def do(name, shape, dt):
        d[name] = nc.dram_tensor(name, shape, dt, kind="ExternalOutput").ap()

    di("xb", [E, S], F32R)            # x[batch].T, own half first
    for w in ("wq", "wk", "wv", "wo"):
        di(w, [E, E], F32R)
    for b in ("bq", "bk", "bo"):
        di(b, [E, 1], F32)
    di("bvb", [128, E], F32)          # bv broadcast across partitions
    di("ln1_g", [E, 1], F32); di("ln1_b", [E, 1], F32)
    di("ln2_g", [E, 1], F32); di("ln2_b", [E, 1], F32)
    di("sw1", [E, DFF], BF16); di("sb1", [DFF, 1], F32)
    di("sw2", [DFF, E], BF16); di("sb2", [E, 1], F32)
    di("wg", [E, NE], F32)
    di("onesd", [128, 128], F32R)
    do("partial", [E, TOK], F32)      # x + attn + shared  (FM)
    do("h2out", [E, TOK], BF16)
    do("logits", [NE, TOK], F32)

    with tile.TileContext(nc) as tc, ExitStack() as ctx:
        pacc = ctx.enter_context(tc.tile_pool(name="pacc", bufs=8, space="PSUM"))
        const = ctx.enter_context(tc.tile_pool(name="const", bufs=1))
        wpool = ctx.enter_context(tc.tile_pool(name="wpool", bufs=3))
        stat = ctx.enter_context(tc.tile_pool(name="stat", bufs=2))
        tmp = ctx.enter_context(tc.tile_pool(name="tmp", bufs=2))

        ones = const.tile([128, 128], F32R, tag="ones")
        nc.sync.dma_start(ones[:], d["onesd"][:])

        def bias_tiles(name, n, tag):
            t = const.tile([128, n], F32, tag=tag, name=f"bt_{tag}")
            nc.sync.dma_start(t[:].rearrange("p (k o) -> p k o", o=1),
                              d[name].rearrange("(k p) o -> p k o", p=128))
            return [t[:, k:k + 1] for k in range(n)]

        g1 = bias_tiles("ln1_g", NK, "g1"); b1 = bias_tiles("ln1_b", NK, "b1")

        def layernorm(src_r, src_f, gt, bt, ncols, outs, chunk_sel=None):
            """src_r(k, sl)->fp32r AP (matmul rhs / Square in);
            src_f(k, sl)->fp32 AP; outs(k, c)->dest AP [128,512]."""
            nch = ncols // 512
            chunks = chunk_sel if chunk_sel is not None else range(nch)
            for c in chunks:
                sl = slice(c * 512, c * 512 + 512)
                ms = pacc.tile([1, 512], F32, tag="mm", name="ms")
                qs = pacc.tile([1, 512], F32, tag="mm", name="qs")
                for k in range(NK):
                    sr = src_r(k, sl)
                    nc.tensor.matmul(ms[:], ones[:, 0:1], sr,
                                     start=(k == 0), stop=(k == NK - 1))
                    xsq = tmp.tile([128, 512], F32R, tag="xsq", name="xsq")
                    nc.scalar.activation(xsq[:], sr, AF.Square)
                    nc.tensor.matmul(qs[:], ones[:, 0:1], xsq[:],
                                     start=(k == 0), stop=(k == NK - 1))
                m_sb = stat.tile([1, 512], F32, tag="scr", bufs=6, name="m_sb")
                nc.scalar.activation(m_sb[:], ms[:], AF.Copy, scale=1.0 / E)
                mq_sb = stat.tile([1, 512], F32, tag="scr", bufs=6, name="mq_sb")
                nc.scalar.activation(mq_sb[:], qs[:], AF.Copy, scale=1.0 / E)
                m2 = stat.tile([1, 512], F32, tag="scr", bufs=6, name="m2")
                nc.scalar.activation(m2[:], m_sb[:], AF.Square)
                var = stat.tile([1, 512], F32, tag="scr", bufs=6, name="var")
                nc.vector.tensor_sub(var[:], mq_sb[:], m2[:])
                nc.vector.tensor_scalar_add(var[:], var[:], EPS)
                std = stat.tile([1, 512], F32, tag="scr", bufs=6, name="std")
                nc.scalar.activation(std[:], var[:], AF.Sqrt)
                rstd = stat.tile([1, 512], F32, tag="scr", bufs=6, name="rstd")
                nc.vector.reciprocal(rstd[:], std[:])
                m_r = stat.tile([1, 512], F32R, tag="scr", bufs=6, name="m_r")
                nc.vector.tensor_copy(m_r[:], m_sb[:])
                r_r = stat.tile([1, 512], F32R, tag="scr", bufs=6, name="r_r")
                nc.vector.tensor_copy(r_r[:], rstd[:])
                mb = pacc.tile([128, 512], F32, tag="mm", name="mb")
                nc.tensor.matmul(mb[:], ones[0:1, :], m_r[:], start=True, stop=True)
                rb = pacc.tile([128, 512], F32, tag="mm", name="rb")
                nc.tensor.matmul(rb[:], ones[0:1, :], r_r[:], start=True, stop=True)
                for k in range(NK):
                    t1 = tmp.tile([128, 512], F32, tag="t1", bufs=3, name="t1")
                    nc.vector.tensor_sub(t1[:], src_f(k, sl), mb[:])
                    nc.vector.tensor_mul(t1[:], t1[:], rb[:])
                    nc.vector.tensor_scalar(outs(k, c), t1[:], gt[k][:],
                                            bt[k][:], mybir.AluOpType.mult,
                                            mybir.AluOpType.add)

        cpool = ctx.enter_context(tc.tile_pool(name="cpool", bufs=1))
        if True:
            s_attn = ExitStack()
            qpool = s_attn.enter_context(tc.tile_pool(name="qpool", bufs=1))
            kpool = s_attn.enter_context(tc.tile_pool(name="kpool", bufs=1))
            vpool = s_attn.enter_context(tc.tile_pool(name="vpool", bufs=1))

            with tc.tile_pool(name="h1pool", bufs=1) as h1pool:
                h1 = [h1pool.tile([128, S], F32R, tag=f"h1{k}", name=f"h1{k}")
                      for k in range(NK)]

                with tc.tile_pool(name="xstrp", bufs=3) as xstrp:
                    def ln1_srcr(k, sl):
                        t = xstrp.tile([128, 512], F32R, tag="xstr",
                                       name="xstr")
                        nc.sync.dma_start(t[:],
                                          d["xb"][k * 128:(k + 1) * 128, sl])
                        return t[:]

                    def ln1_srcf(k, sl):
                        t = xstrp.tile([128, 512], F32R, tag="xstr",
                                       name="xstr2")
                        nc.sync.dma_start(t[:],
                                          d["xb"][k * 128:(k + 1) * 128, sl])
                        return t[:].bitcast(F32)

                    layernorm(ln1_srcr, ln1_srcf, g1, b1, S,
                              lambda k, c: h1[k][:, c * 512:(c + 1) * 512],
                              chunk_sel=[0])

                    g2 = bias_tiles("ln2_g", NK, "g2")
                    b2 = bias_tiles("ln2_b", NK, "b2")
                    bq = bias_tiles("bq", NK, "bq")
                    bk = bias_tiles("bk", NK, "bk")
                    bo = bias_tiles("bo", NK, "bo")
                    sb1t = bias_tiles("sb1", DFF // 128, "sb1")
                    sb2t = bias_tiles("sb2", NK, "sb2")
                    bvb_sb = const.tile([128, E], F32, tag="bvb")
                    nc.sync.dma_start(bvb_sb[:], d["bvb"][:])

                    # ---- Q (own 512 tokens) — overlaps LN1 chunk 1 ----
                    qps = [pacc.tile([128, 512], F32, tag="mm", name=f"qps{i}")
                           for i in range(NK)]
                    for k in range(NK):
                        wt = wpool.tile([128, E], F32R, tag="w", bufs=4,
                                        name="wqk")
                        nc.sync.dma_start(wt[:], d["wq"][k * 128:(k + 1) * 128, :])
                        for m in range(NK):
                            nc.tensor.matmul(qps[m][:],
                                             wt[:, m * 128:(m + 1) * 128],
                                             h1[k][:, 0:512], start=(k == 0),
                                             stop=(k == NK - 1))
                    qsb = []
                    for m in range(NK):
                        t = qpool.tile([128, 512], F32R, tag=f"q{m}",
                                       name=f"q{m}")
                        nc.vector.tensor_scalar_add(t[:], qps[m][:], bq[m][:])
                        qsb.append(t)

                    layernorm(ln1_srcr, ln1_srcf, g1, b1, S,
                              lambda k, c: h1[k][:, c * 512:(c + 1) * 512],
                              chunk_sel=[1])

                # ---- K (all 1024 tokens, two chunk passes) ----
                ksb = [kpool.tile([128, S], F32R, tag=f"k{m}", name=f"ksb{m}")
                       for m in range(NK)]
                for c2 in range(2):
                    kps = [pacc.tile([128, 512], F32, tag="mm", name=f"kps{i}")
                           for i in range(NK)]
                    for k in range(NK):
                        wt = wpool.tile([128, E], F32R, tag="w", bufs=4, name="wkk")
                        nc.sync.dma_start(wt[:], d["wk"][k * 128:(k + 1) * 128, :])
                        for m in range(NK):
                            nc.tensor.matmul(kps[m][:],
                                             wt[:, m * 128:(m + 1) * 128],
                                             h1[k][:, c2 * 512:(c2 + 1) * 512],
                                             start=(k == 0), stop=(k == NK - 1))
                    for m in range(NK):
                        nc.vector.tensor_scalar_add(
                            ksb[m][:, c2 * 512:(c2 + 1) * 512],
                            kps[m][:], bk[m][:])

                # ---- V (token-major, 65-strided per head, ones col) ----
                vsb = [vpool.tile([128, 16 * 65], F32R, tag=f"v{t}",
                                  name=f"vsb{t}") for t in range(NK)]
                for t in range(NK):
                    ov = vsb[t][:].rearrange("p (h e) -> p h e", e=65)[:, :, 64:65]
                    nc.scalar.copy(ov, ones[:, 0:16].rearrange(
                        "p (h e) -> p h e", e=1))
                for tg in range(2):
                    vps = {}
                    for t in range(4):
                        for f in range(2):
                            vps[(t, f)] = pacc.tile([128, 512], F32, tag="mm",
                                                    name=f"vps{t}_{f}")
                    for k in range(NK):
                        wt = wpool.tile([128, E], F32R, tag="w", bufs=4, name="wvk")
                        nc.sync.dma_start(wt[:], d["wv"][k * 128:(k + 1) * 128, :])
                        for t in range(4):
                            tt = tg * 4 + t
                            for f in range(2):
                                nc.tensor.matmul(
                                    vps[(t, f)][:],
                                    h1[k][:, tt * 128:tt * 128 + 128],
                                    wt[:, f * 512:(f + 1) * 512],
                                    start=(k == 0), stop=(k == NK - 1))
                    for t in range(4):
                        for f in range(2):
                            dst = vsb[tg * 4 + t][:, f * 520:(f + 1) * 520] \
                                .rearrange("p (h e) -> p h e", e=65)[:, :, 0:64]
                            src = vps[(t, f)][:].rearrange("p (h e) -> p h e", e=64)
                            nc.vector.tensor_add(
                                dst, src,
                                bvb_sb[:, f * 512:(f + 1) * 512].rearrange(
                                    "p (h e) -> p h e", e=64))
            # h1 freed here

            # ---- attention per head ----
            ppool = s_attn.enter_context(tc.tile_pool(name="ppool", bufs=6))
            stat2 = s_attn.enter_context(tc.tile_pool(name="stat2", bufs=2))
            packed = [cpool.tile([128, 512], F32R, tag=f"c{p}", name=f"packed{p}")
                      for p in range(NK)]
            LOOKAHEAD = 2
            pending_norm = []
            for hp in range(H // 2):
                heads = (2 * hp, 2 * hp + 1)
                cps = {}
                for h in heads:
                    cps[h] = pacc.tile([65, 512], F32, tag="mm", name=f"cps{h}")

                pend = {}  # kc -> {h: exp tile}

                def emit_s(kc):
                    psbs = {}
                    for h in heads:
                        ktile = ksb[hp][(h % 2) * 64:(h % 2) * 64 + 64, :]
                        qtile = qsb[hp][(h % 2) * 64:(h % 2) * 64 + 64, :]
                        sps = pacc.tile([128, 512], F32, tag="mm",
                                        name=f"sps{h}_{kc}")
                        nc.tensor.matmul(sps[:],
                                         ktile[:, kc * 128:kc * 128 + 128],
                                         qtile[:], start=True, stop=True)
                        psb = ppool.tile([128, 512], F32R, tag="p",
                                         name=f"p{h}_{kc}")
                        nc.scalar.activation(psb[:], sps[:], AF.Exp,
                                             scale=1.0 / math.sqrt(DH))
                        psbs[h] = psb
                    pend[kc] = psbs

                def emit_pv(kc):
                    psbs = pend.pop(kc)
                    for h in heads:
                        nc.tensor.matmul(cps[h][:],
                                         vsb[kc][:, h * 65:h * 65 + 65],
                                         psbs[h][:], start=(kc == 0),
                                         stop=(kc == NK - 1))

                for kc in range(NK):
                    emit_s(kc)
                    if kc >= LOOKAHEAD:
                        emit_pv(kc - LOOKAHEAD)
                for kc in range(NK - LOOKAHEAD, NK):
                    emit_pv(kc)

                def normalize(hp=hp, cps=cps, heads=heads):
                    for h in heads:
                        rd = stat2.tile([1, 512], F32, tag="rd", name="rd")
                        nc.vector.reciprocal(rd[:], cps[h][64:65, :])
                        rdr = stat2.tile([1, 512], F32R, tag="rdr", name="rdr")
                        nc.vector.tensor_copy(rdr[:], rd[:])
                        bcp = pacc.tile([64, 512], F32, tag="mm", name=f"bcp{h}")
                        nc.tensor.matmul(bcp[:], ones[0:1, 0:64], rdr[:],
                                         start=True, stop=True)
                        bcs = tmp.tile([64, 512], F32, tag="sf", bufs=4,
                                       name="bcs")
                        nc.vector.tensor_copy(bcs[:], bcp[:])
                        if h % 2 == 0:
                            nc.vector.tensor_mul(packed[hp][0:64, :],
                                                 cps[h][0:64, :], bcs[:])
                        else:
                            ct = tmp.tile([64, 512], F32R, tag="sf", bufs=4,
                                          name="ct")
                            nc.vector.tensor_mul(ct[:], cps[h][0:64, :], bcs[:])
                            nc.sync.dma_start(packed[hp][64:128, :], ct[:])

                pending_norm.append(normalize)
                if len(pending_norm) > 1:
                    pending_norm.pop(0)()
            while pending_norm:
                pending_norm.pop(0)()
            s_attn.close()  # q/k/v/p freed

            # ---- O-proj + residual ----
            x2pool = ctx.enter_context(tc.tile_pool(name="x2pool", bufs=1))
            ops = [pacc.tile([128, 512], F32, tag="mm", name=f"ops{i}")
                   for i in range(NK)]
            for k in range(NK):
                wt = wpool.tile([128, E], F32R, tag="w", bufs=4, name="wok")
                nc.sync.dma_start(wt[:], d["wo"][k * 128:(k + 1) * 128, :])
                for m in range(NK):
                    nc.tensor.matmul(ops[m][:], wt[:, m * 128:(m + 1) * 128],
                                     packed[k][:], start=(k == 0),
                                     stop=(k == NK - 1))
            x2 = []
            x2r = []
            for m in range(NK):
                xot = tmp.tile([128, 512], F32, tag="sf", bufs=4, name="xot")
                nc.sync.dma_start(xot[:], d["xb"][m * 128:(m + 1) * 128, 0:512]
                                  .bitcast(F32))
                osb = tmp.tile([128, 512], F32, tag="sf", bufs=4, name="osb")
                nc.vector.tensor_scalar_add(osb[:], ops[m][:], bo[m][:])
                t = x2pool.tile([128, 512], F32, tag=f"x2{m}", name=f"x2_{m}")
                nc.vector.tensor_add(t[:], osb[:], xot[:])
                x2.append(t)
                tr = x2pool.tile([128, 512], F32R, tag=f"x2r{m}", name=f"x2r{m}")
                nc.vector.tensor_copy(tr[:], t[:])
                x2r.append(tr)
        # xown, packed freed

        # ---- LN2 ----
        h2pool = ctx.enter_context(tc.tile_pool(name="h2pool", bufs=1))
        outp = ctx.enter_context(tc.tile_pool(name="outp", bufs=2))
        h2f = [h2pool.tile([128, 512], F32, tag=f"h2f{k}", name=f"h2f{k}")
               for k in range(NK)]
        layernorm(lambda k, sl: x2r[k][:, sl], lambda k, sl: x2[k][:, sl],
                  g2, b2, 512, lambda k, c: h2f[k][:])
        h2b = []
        for k in range(NK):
            t = h2pool.tile([128, 512], BF16, tag=f"h2b{k}", name=f"h2b{k}")
            nc.vector.tensor_copy(t[:], h2f[k][:])
            h2b.append(t)
            nc.sync.dma_start(d["h2out"][k * 128:(k + 1) * 128, :], t[:])

        # ---- router logits (full fp32) ----
        wgt = const.tile([128, NE * NK], F32, tag="wg")
        nc.sync.dma_start(wgt[:].rearrange("p (k e) -> p k e", e=NE),
                          d["wg"].rearrange("(k p) e -> p k e", p=128))
        gps = pacc.tile([NE, 512], F32, tag="mm", name="gps")
        for k in range(NK):
            nc.tensor.matmul(gps[:], wgt[:, k * NE:(k + 1) * NE], h2f[k][:],
                             start=(k == 0), stop=(k == NK - 1))
        lsb = outp.tile([NE, 512], F32, tag="l", name="lsb")
        nc.vector.tensor_copy(lsb[:], gps[:])
        nc.sync.dma_start(d["logits"][:], lsb[:])

        # ---- shared MLP (bf16) ----
        with tc.tile_pool(name="midpool", bufs=1) as midpool:
            mid = []
            for jg in range(4):
                mps = [pacc.tile([128, 512], F32, tag="mm", name=f"mps{jg}_{i}")
                       for i in range(8)]
                for k in range(NK):
                    wt = wpool.tile([128, 1024], BF16, tag="wb", bufs=8, name="sw1k")
                    nc.sync.dma_start(wt[:], d["sw1"][k * 128:(k + 1) * 128,
                                                      jg * 1024:(jg + 1) * 1024])
                    for j in range(8):
                        nc.tensor.matmul(mps[j][:], wt[:, j * 128:(j + 1) * 128],
                                         h2b[k][:], start=(k == 0),
                                         stop=(k == NK - 1))
                for j in range(8):
                    jj = jg * 8 + j
                    t = midpool.tile([128, 512], BF16, tag=f"mid{jj}",
                                     name=f"mid{jj}")
                    nc.scalar.activation(t[:], mps[j][:], AF.Gelu,
                                         bias=sb1t[jj][:])
                    mid.append(t)
            for mg in range(2):
                o2 = [pacc.tile([128, 512], F32, tag="mm", name=f"o2_{i}")
                      for i in range(4)]
                for j in range(DFF // 128):
                    wt = wpool.tile([128, 512], BF16, tag="wb", bufs=8,
                                    name="sw2j")
                    nc.sync.dma_start(wt[:], d["sw2"][j * 128:(j + 1) * 128,
                                                      mg * 512:(mg + 1) * 512])
                    for m in range(4):
                        nc.tensor.matmul(o2[m][:], wt[:, m * 128:(m + 1) * 128],
                                         mid[j][:], start=(j == 0),
                                         stop=(j == DFF // 128 - 1))
                for m in range(4):
                    mm2 = mg * 4 + m
                    sh = tmp.tile([128, 512], F32, tag="sf", bufs=4, name="sh")
                    nc.vector.tensor_scalar_add(sh[:], o2[m][:], sb2t[mm2][:])
                    pt = outp.tile([128, 512], F32, tag="pt", name="pt")
                    nc.vector.tensor_add(pt[:], sh[:], x2[mm2][:])
                    nc.sync.dma_start(d["partial"][mm2 * 128:(mm2 + 1) * 128, :],
                                      pt[:])

    nc.compile()
    return nc


def _build_launch_b(chunks):
    nc = bacc.Bacc("TRN2", target_bir_lowering=False, debug=False,
                   enable_asserts=True, num_devices=NCORES)
    C = sum(chunks)
    d = {}
    d["h2d"] = nc.dram_tensor("h2d", [E, C], BF16, kind="ExternalInput").ap()
    d["e1"] = nc.dram_tensor("e1", [E, DFF], BF16, kind="ExternalInput").ap()
    d["e2"] = nc.dram_tensor("e2", [DFF, E], BF16, kind="ExternalInput").ap()
    d["b1"] = nc.dram_tensor("b1", [DFF, 1], F32, kind="ExternalInput").ap()
    d["b2"] = nc.dram_tensor("b2", [E, 1], F32, kind="ExternalInput").ap()
    d["yout"] = nc.dram_tensor("yout", [E, C], F32, kind="ExternalOutput").ap()

    with tile.TileContext(nc) as tc, ExitStack() as ctx:
        pacc = ctx.enter_context(tc.tile_pool(name="pacc", bufs=8, space="PSUM"))
        const = ctx.enter_context(tc.tile_pool(name="const", bufs=1))
        hpool = ctx.enter_context(tc.tile_pool(name="hpool", bufs=1))
        wpool = ctx.enter_context(tc.tile_pool(name="wpool", bufs=14))
        midpool = ctx.enter_context(tc.tile_pool(name="midpool", bufs=1))
        ypool = ctx.enter_context(tc.tile_pool(name="ypool", bufs=2))

        b1w = const.tile([128, DFF // 128], F32, tag="b1w", name="b1w")
        nc.sync.dma_start(b1w[:].rearrange("p (k o) -> p k o", o=1),
                          d["b1"].rearrange("(k p) o -> p k o", p=128))
        b1t = [b1w[:, k:k + 1] for k in range(DFF // 128)]
        b2w = const.tile([128, NK], F32, tag="b2w", name="b2w")
        nc.sync.dma_start(b2w[:].rearrange("p (k o) -> p k o", o=1),
                          d["b2"].rearrange("(k p) o -> p k o", p=128))
        b2t = [b2w[:, k:k + 1] for k in range(NK)]
        h2d = [hpool.tile([128, C], BF16, tag=f"h{k}", name=f"h2d{k}")
               for k in range(NK)]

        off = 0
        for ci, cw in enumerate(chunks):
            csl = slice(off, off + cw)
            mid = []
            for jg in range(4):
                mps = [pacc.tile([128, cw], F32, tag="mm", name=f"bmps{i}")
                       for i in range(8)]
                for k in range(NK):
                    wt = wpool.tile([128, 1024], BF16, tag="w1", name="wt")
                    nc.sync.dma_start(wt[:], d["e1"][k * 128:(k + 1) * 128,
                                                     jg * 1024:(jg + 1) * 1024])
                    if ci == 0 and jg == 0:
                        nc.sync.dma_start(h2d[k][:],
                                          d["h2d"][k * 128:(k + 1) * 128, :])
                    for j in range(8):
                        nc.tensor.matmul(mps[j][:], wt[:, j * 128:(j + 1) * 128],
                                         h2d[k][:, csl], start=(k == 0),
                                         stop=(k == NK - 1))
                for j in range(8):
                    jj = jg * 8 + j
                    t = midpool.tile([128, cw], BF16, tag=f"mid{jj}_{ci % 2}",
                                     name=f"bmid{jj}")
                    nc.scalar.activation(t[:], mps[j][:], AF.Gelu,
                                         bias=b1t[jj][:])
                    mid.append(t)
            o2 = [pacc.tile([128, cw], F32, tag="mm", name=f"bo2_{i}")
                  for i in range(NK)]
            for j in range(DFF // 128):
                wt = wpool.tile([128, 1024], BF16, tag="w2", name="wt2")
                nc.sync.dma_start(wt[:], d["e2"][j * 128:(j + 1) * 128, :])
                for m in range(NK):
                    nc.tensor.matmul(o2[m][:], wt[:, m * 128:(m + 1) * 128],
                                     mid[j][:], start=(j == 0),
                                     stop=(j == DFF // 128 - 1))
            for m in range(NK):
                y = ypool.tile([128, cw], F32, tag="y", name="y")
                nc.scalar.activation(y[:], o2[m][:], AF.Identity,
                                     bias=b2t[m][:])
                nc.sync.dma_start(d["yout"][m * 128:(m + 1) * 128, csl], y[:])
            off += cw

    nc.compile()
    return nc


def _chunk_sizes(C):
    n = (C + 511) // 512
    base = C // n // 8 * 8
    sizes = [base] * n
    rem = C - base * n
    i = 0
    while rem > 0:
        step = min(8, rem)
        sizes[i] += step
        rem -= step
        i = (i + 1) % n
    assert sum(sizes) == C and all(s <= 512 for s in sizes)
    return sizes


def kernel(**inputs):
    global last_results, last_programs
    last_results = []
    last_programs = []

    f32 = lambda a: np.ascontiguousarray(np.asarray(a), dtype=np.float32)
    x = f32(inputs["x"])
    col = lambda a: f32(a).reshape(-1, 1)

    if "A" not in _cache:
        _cache["A"] = _build_launch_a()
    ncA = _cache["A"]

    wq, wk, wv, wo = (f32(inputs[k]) for k in ("wq", "wk", "wv", "wo"))
    sw1 = f32(inputs["sw1"]).astype(ml_dtypes.bfloat16)
    sw2 = f32(inputs["sw2"]).astype(ml_dtypes.bfloat16)
    onesd = np.ones((128, 128), np.float32)
    bvb = np.broadcast_to(f32(inputs["bv"]), (128, E)).copy()
    shared_in = dict(
        wq=wq, wk=wk, wv=wv, wo=wo,
        bq=col(inputs["bq"]), bk=col(inputs["bk"]), bo=col(inputs["bo"]),
        bvb=bvb,
        ln1_g=col(inputs["ln1_g"]), ln1_b=col(inputs["ln1_b"]),
        ln2_g=col(inputs["ln2_g"]), ln2_b=col(inputs["ln2_b"]),
        sw1=sw1, sb1=col(inputs["sb1"]), sw2=sw2, sb2=col(inputs["sb2"]),
        wg=f32(inputs["w_gate"]), onesd=onesd,
    )
    in_maps = []
    for c in range(NCORES):
        b, half = c // 2, c % 2
        xt = x[b].T  # [E, S]
        own = xt[:, half * 512:(half + 1) * 512]
        oth = xt[:, (1 - half) * 512:(2 - half) * 512]
        xb = np.ascontiguousarray(np.concatenate([own, oth], axis=1))
        in_maps.append({**shared_in, "xb": xb})

    resA = run_bass_kernel_spmd(ncA, in_maps, core_ids=list(range(NCORES)))
    last_results.append(("A", resA))
    last_programs.append(("A", ncA))

    partial = np.concatenate([resA.results[c]["partial"].T for c in range(NCORES)], 0)
    h2bf = np.concatenate([resA.results[c]["h2out"] for c in range(NCORES)], 1)
    logits = np.concatenate([resA.results[c]["logits"].T for c in range(NCORES)], 0)

    # ---- routing on host (mirrors reference, fp32) ----
    N = B * S
    order = np.argsort(-logits, axis=-1, kind="stable")
    top_idx = order[:, :TOPK]
    top_vals = np.take_along_axis(logits, top_idx, axis=-1)
    tv = top_vals - top_vals.max(-1, keepdims=True)
    te = np.exp(tv, dtype=np.float32)
    top_gates = te / te.sum(-1, keepdims=True)
    gates_dense = np.zeros((N, NE), np.float32)
    np.put_along_axis(gates_dense, top_idx, top_gates, axis=-1)
    lm = logits - logits.max(-1, keepdims=True)
    le = np.exp(lm, dtype=np.float32)
    probs = le / le.sum(-1, keepdims=True)
    P = probs.mean(0, dtype=np.float32)
    f = (gates_dense > 0).astype(np.float32).mean(0, dtype=np.float32)
    aux = np.float32(NE * np.sum(P * f, dtype=np.float32))

    # ---- dispatch ----
    sel_lists = [np.nonzero((top_idx == e).any(-1))[0] for e in range(NE)]
    counts = np.array([len(t) for t in sel_lists])
    C = int(max(512, -(-counts.max() // 8) * 8))
    chunks = tuple(_chunk_sizes(C))
    key = ("B", chunks)
    if key not in _cache:
        _cache[key] = _build_launch_b(list(chunks))
    ncB = _cache[key]

    in_maps_b = []
    ew1 = np.asarray(inputs["ew1"]).astype(ml_dtypes.bfloat16)
    ew2 = np.asarray(inputs["ew2"]).astype(ml_dtypes.bfloat16)
    eb1 = f32(inputs["eb1"]); eb2 = f32(inputs["eb2"])
    idxs = []
    for e in range(NE):
        idx = np.zeros(C, np.int64)
        idx[:counts[e]] = sel_lists[e]
        idxs.append(idx)
        h2d = np.ascontiguousarray(h2bf[:, idx])
        in_maps_b.append(dict(h2d=h2d, e1=np.ascontiguousarray(ew1[e]),
                              e2=np.ascontiguousarray(ew2[e]),
                              b1=eb1[e].reshape(-1, 1).astype(np.float32),
                              b2=eb2[e].reshape(-1, 1).astype(np.float32)))
    resB = run_bass_kernel_spmd(ncB, in_maps_b, core_ids=list(range(NCORES)))
    last_results.append(("B", resB))
    last_programs.append(("B", ncB))

    out = partial
    for e in range(NE):
        cnt = counts[e]
        if cnt == 0:
            continue
        y = resB.results[e]["yout"][:, :cnt].T  # [cnt, E]
        g = gates_dense[idxs[e][:cnt], e][:, None]
        out[idxs[e][:cnt]] += g * y
    return out.reshape(B, S, E).astype(np.float32), aux


# revision 42
# speedup vs baseline: 1.0536x; 1.0033x over previous
"""MoE transformer block on 8 TRN2 NeuronCores.

Launch A (data-parallel over tokens): per core = (batch b, seq half) -> 512
query tokens.  Attention path in fp32r (router-accuracy critical), shared MLP
in bf16.  Outputs partial = x2 + shared, h2 (bf16), router logits (fp32).

Host: top-2 routing, gates, aux loss, per-expert token dispatch.

Launch B (expert-parallel): core e runs expert e's MLP over its C dispatched
tokens, bf16 weights/activations.

Host: gated combine -> full output.
"""

import math
import numpy as np
import ml_dtypes
from contextlib import ExitStack

import concourse.bass as bass
import concourse.tile as tile
from concourse import bacc, mybir
from concourse.bass_utils import run_bass_kernel_spmd

F32 = mybir.dt.float32
F32R = mybir.dt.float32r
BF16 = mybir.dt.bfloat16
AF = mybir.ActivationFunctionType

B, S, E, H, NE, TOPK = 4, 1024, 1024, 16, 8, 2
DFF = 4 * E
DH = E // H
EPS = 1e-5
NCORES = 8
TOK = 512          # own tokens per core in launch A
NK = E // 128      # 8 feature tiles

_cache = {}
last_results = []   # [(name, BassKernelResults), ...] for test harness
last_programs = []  # [(name, Bacc), ...] for test harness timing


def _build_launch_a(trace=False):
    nc = bacc.Bacc("TRN2", target_bir_lowering=False, debug=False,
                   enable_asserts=True, num_devices=NCORES)
    d = {}
    def di(name, shape, dt):
        d[name] = nc.dram_tensor(name, shape, dt, kind="ExternalInput").ap()
    def do(name, shape, dt):
        d[name] = nc.dram_tensor(name, shape, dt, kind="ExternalOutput").ap()

    di("xb", [E, S], F32R)            # x[batch].T, own half first
    for w in ("wq", "wk", "wv", "wo"):
        di(w, [E, E], F32R)
    for b in ("bq", "bk", "bo"):
        di(b, [E, 1], F32)
    di("bvb", [128, E], F32)          # bv broadcast across partitions
    di("ln1_g", [E, 1], F32); di("ln1_b", [E, 1], F32)
    di("ln2_g", [E, 1], F32); di("ln2_b", [E, 1], F32)
    di("sw1", [E, DFF], BF16); di("sb1", [DFF, 1], F32)
    di("sw2", [DFF, E], BF16); di("sb2", [E, 1], F32)
    di("wg", [E, NE], F32)
    di("onesd", [128, 128], F32R)
    do("partial", [E, TOK], F32)      # x + attn + shared  (FM)
    do("h2out", [E, TOK], BF16)
    do("logits", [NE, TOK], F32)

    with tile.TileContext(nc) as tc, ExitStack() as ctx:
        pacc = ctx.enter_context(tc.tile_pool(name="pacc", bufs=8, space="PSUM"))
        const = ctx.enter_context(tc.tile_pool(name="const", bufs=1))
        wpool = ctx.enter_context(tc.tile_pool(name="wpool", bufs=3))
        stat = ctx.enter_context(tc.tile_pool(name="stat", bufs=2))
        tmp = ctx.enter_context(tc.tile_pool(name="tmp", bufs=2))

        ones = const.tile([128, 128], F32R, tag="ones")
        nc.sync.dma_start(ones[:], d["onesd"][:])

        def bias_tiles(name, n, tag):
            t = const.tile([128, n], F32, tag=tag, name=f"bt_{tag}")
            nc.sync.dma_start(t[:].rearrange("p (k o) -> p k o", o=1),
                              d[name].rearrange("(k p) o -> p k o", p=128))
            return [t[:, k:k + 1] for k in range(n)]

        g1 = bias_tiles("ln1_g", NK, "g1"); b1 = bias_tiles("ln1_b", NK, "b1")

        def layernorm(src_r, src_f, gt, bt, ncols, outs, chunk_sel=None):
            """src_r(k, sl)->fp32r AP (matmul rhs / Square in);
            src_f(k, sl)->fp32 AP; outs(k, c)->dest AP [128,512]."""
            nch = ncols // 512
            chunks = chunk_sel if chunk_sel is not None else range(nch)
            for c in chunks:
                sl = slice(c * 512, c * 512 + 512)
                ms = pacc.tile([1, 512], F32, tag="mm", name="ms")
                qs = pacc.tile([1, 512], F32, tag="mm", name="qs")
                for k in range(NK):
                    sr = src_r(k, sl)
                    nc.tensor.matmul(ms[:], ones[:, 0:1], sr,
                                     start=(k == 0), stop=(k == NK - 1))
                    xsq = tmp.tile([128, 512], F32R, tag="xsq", name="xsq")
                    nc.scalar.activation(xsq[:], sr, AF.Square)
                    nc.tensor.matmul(qs[:], ones[:, 0:1], xsq[:],
                                     start=(k == 0), stop=(k == NK - 1))
                m_sb = stat.tile([1, 512], F32, tag="scr", bufs=6, name="m_sb")
                nc.scalar.activation(m_sb[:], ms[:], AF.Copy, scale=1.0 / E)
                mq_sb = stat.tile([1, 512], F32, tag="scr", bufs=6, name="mq_sb")
                nc.scalar.activation(mq_sb[:], qs[:], AF.Copy, scale=1.0 / E)
                m2 = stat.tile([1, 512], F32, tag="scr", bufs=6, name="m2")
                nc.scalar.activation(m2[:], m_sb[:], AF.Square)
                var = stat.tile([1, 512], F32, tag="scr", bufs=6, name="var")
                nc.vector.tensor_sub(var[:], mq_sb[:], m2[:])
                nc.vector.tensor_scalar_add(var[:], var[:], EPS)
                std = stat.tile([1, 512], F32, tag="scr", bufs=6, name="std")
                nc.scalar.activation(std[:], var[:], AF.Sqrt)
                rstd = stat.tile([1, 512], F32, tag="scr", bufs=6, name="rstd")
                nc.vector.reciprocal(rstd[:], std[:])
                m_r = stat.tile([1, 512], F32R, tag="scr", bufs=6, name="m_r")
                nc.vector.tensor_copy(m_r[:], m_sb[:])
                r_r = stat.tile([1, 512], F32R, tag="scr", bufs=6, name="r_r")
                nc.vector.tensor_copy(r_r[:], rstd[:])
                mb = pacc.tile([128, 512], F32, tag="mm", name="mb")
                nc.tensor.matmul(mb[:], ones[0:1, :], m_r[:], start=True, stop=True)
                rb = pacc.tile([128, 512], F32, tag="mm", name="rb")
                nc.tensor.matmul(rb[:], ones[0:1, :], r_r[:], start=True, stop=True)
                for k in range(NK):
                    t1 = tmp.tile([128, 512], F32, tag="t1", bufs=3, name="t1")
                    nc.vector.tensor_sub(t1[:], src_f(k, sl), mb[:])
                    nc.vector.tensor_mul(t1[:], t1[:], rb[:])
                    nc.vector.tensor_scalar(outs(k, c), t1[:], gt[k][:],
                                            bt[k][:], mybir.AluOpType.mult,
                                            mybir.AluOpType.add)

        cpool = ctx.enter_context(tc.tile_pool(name="cpool", bufs=1))
        if True:
            s_attn = ExitStack()
            qpool = s_attn.enter_context(tc.tile_pool(name="qpool", bufs=1))
            kpool = s_attn.enter_context(tc.tile_pool(name="kpool", bufs=1))
            vpool = s_attn.enter_context(tc.tile_pool(name="vpool", bufs=1))

            with tc.tile_pool(name="h1pool", bufs=1) as h1pool:
                h1 = [h1pool.tile([128, S], F32R, tag=f"h1{k}", name=f"h1{k}")
                      for k in range(NK)]

                with tc.tile_pool(name="xstrp", bufs=3) as xstrp:
                    def ln1_srcr(k, sl):
                        t = xstrp.tile([128, 512], F32R, tag="xstr",
                                       name="xstr")
                        nc.sync.dma_start(t[:],
                                          d["xb"][k * 128:(k + 1) * 128, sl])
                        return t[:]

                    def ln1_srcf(k, sl):
                        t = xstrp.tile([128, 512], F32R, tag="xstr",
                                       name="xstr2")
                        nc.sync.dma_start(t[:],
                                          d["xb"][k * 128:(k + 1) * 128, sl])
                        return t[:].bitcast(F32)

                    layernorm(ln1_srcr, ln1_srcf, g1, b1, S,
                              lambda k, c: h1[k][:, c * 512:(c + 1) * 512],
                              chunk_sel=[0])

                    g2 = bias_tiles("ln2_g", NK, "g2")
                    b2 = bias_tiles("ln2_b", NK, "b2")
                    bq = bias_tiles("bq", NK, "bq")
                    bk = bias_tiles("bk", NK, "bk")
                    bo = bias_tiles("bo", NK, "bo")
                    sb1t = bias_tiles("sb1", DFF // 128, "sb1")
                    sb2t = bias_tiles("sb2", NK, "sb2")
                    bvb_sb = const.tile([128, E], F32, tag="bvb")
                    nc.sync.dma_start(bvb_sb[:], d["bvb"][:])

                    # ---- Q (own 512 tokens) — overlaps LN1 chunk 1 ----
                    qps = [pacc.tile([128, 512], F32, tag="mm", name=f"qps{i}")
                           for i in range(NK)]
                    for k in range(NK):
                        wt = wpool.tile([128, E], F32R, tag="w", bufs=4,
                                        name="wqk")
                        nc.sync.dma_start(wt[:], d["wq"][k * 128:(k + 1) * 128, :])
                        for m in range(NK):
                            nc.tensor.matmul(qps[m][:],
                                             wt[:, m * 128:(m + 1) * 128],
                                             h1[k][:, 0:512], start=(k == 0),
                                             stop=(k == NK - 1))
                    qsb = []
                    for m in range(NK):
                        t = qpool.tile([128, 512], F32R, tag=f"q{m}",
                                       name=f"q{m}")
                        if m % 2 == 0:
                            nc.vector.tensor_scalar_add(t[:], qps[m][:], bq[m][:])
                        else:
                            nc.scalar.activation(t[:], qps[m][:], AF.Identity,
                                                 bias=bq[m][:])
                        qsb.append(t)

                    layernorm(ln1_srcr, ln1_srcf, g1, b1, S,
                              lambda k, c: h1[k][:, c * 512:(c + 1) * 512],
                              chunk_sel=[1])

                # ---- K (all 1024 tokens, two chunk passes) ----
                ksb = [kpool.tile([128, S], F32R, tag=f"k{m}", name=f"ksb{m}")
                       for m in range(NK)]
                for c2 in range(2):
                    kps = [pacc.tile([128, 512], F32, tag="mm", name=f"kps{i}")
                           for i in range(NK)]
                    for k in range(NK):
                        wt = wpool.tile([128, E], F32R, tag="w", bufs=4, name="wkk")
                        nc.sync.dma_start(wt[:], d["wk"][k * 128:(k + 1) * 128, :])
                        for m in range(NK):
                            nc.tensor.matmul(kps[m][:],
                                             wt[:, m * 128:(m + 1) * 128],
                                             h1[k][:, c2 * 512:(c2 + 1) * 512],
                                             start=(k == 0), stop=(k == NK - 1))
                    for m in range(NK):
                        if m % 2 == 0:
                            nc.vector.tensor_scalar_add(
                                ksb[m][:, c2 * 512:(c2 + 1) * 512],
                                kps[m][:], bk[m][:])
                        else:
                            nc.scalar.activation(
                                ksb[m][:, c2 * 512:(c2 + 1) * 512],
                                kps[m][:], AF.Identity, bias=bk[m][:])

                # ---- V (token-major, 65-strided per head, ones col) ----
                vsb = [vpool.tile([128, 16 * 65], F32R, tag=f"v{t}",
                                  name=f"vsb{t}") for t in range(NK)]
                for t in range(NK):
                    ov = vsb[t][:].rearrange("p (h e) -> p h e", e=65)[:, :, 64:65]
                    nc.scalar.copy(ov, ones[:, 0:16].rearrange(
                        "p (h e) -> p h e", e=1))
                for tg in range(2):
                    vps = {}
                    for t in range(4):
                        for f in range(2):
                            vps[(t, f)] = pacc.tile([128, 512], F32, tag="mm",
                                                    name=f"vps{t}_{f}")
                    for k in range(NK):
                        wt = wpool.tile([128, E], F32R, tag="w", bufs=4, name="wvk")
                        nc.sync.dma_start(wt[:], d["wv"][k * 128:(k + 1) * 128, :])
                        for t in range(4):
                            tt = tg * 4 + t
                            for f in range(2):
                                nc.tensor.matmul(
                                    vps[(t, f)][:],
                                    h1[k][:, tt * 128:tt * 128 + 128],
                                    wt[:, f * 512:(f + 1) * 512],
                                    start=(k == 0), stop=(k == NK - 1))
                    for t in range(4):
                        for f in range(2):
                            dst = vsb[tg * 4 + t][:, f * 520:(f + 1) * 520] \
                                .rearrange("p (h e) -> p h e", e=65)[:, :, 0:64]
                            src = vps[(t, f)][:].rearrange("p (h e) -> p h e", e=64)
                            nc.vector.tensor_add(
                                dst, src,
                                bvb_sb[:, f * 512:(f + 1) * 512].rearrange(
                                    "p (h e) -> p h e", e=64))
            # h1 freed here

            # ---- attention per head ----
            ppool = s_attn.enter_context(tc.tile_pool(name="ppool", bufs=6))
            stat2 = s_attn.enter_context(tc.tile_pool(name="stat2", bufs=2))
            packed = [cpool.tile([128, 512], F32R, tag=f"c{p}", name=f"packed{p}")
                      for p in range(NK)]
            LOOKAHEAD = 2
            pending_norm = []
            for hp in range(H // 2):
                heads = (2 * hp, 2 * hp + 1)
                cps = {}
                for h in heads:
                    cps[h] = pacc.tile([65, 512], F32, tag="mm", name=f"cps{h}")

                pend = {}  # kc -> {h: exp tile}

                def emit_s(kc):
                    psbs = {}
                    for h in heads:
                        ktile = ksb[hp][(h % 2) * 64:(h % 2) * 64 + 64, :]
                        qtile = qsb[hp][(h % 2) * 64:(h % 2) * 64 + 64, :]
                        sps = pacc.tile([128, 512], F32, tag="mm",
                                        name=f"sps{h}_{kc}")
                        nc.tensor.matmul(sps[:],
                                         ktile[:, kc * 128:kc * 128 + 128],
                                         qtile[:], start=True, stop=True)
                        psb = ppool.tile([128, 512], F32R, tag="p",
                                         name=f"p{h}_{kc}")
                        nc.scalar.activation(psb[:], sps[:], AF.Exp,
                                             scale=1.0 / math.sqrt(DH))
                        psbs[h] = psb
                    pend[kc] = psbs

                def emit_pv(kc):
                    psbs = pend.pop(kc)
                    for h in heads:
                        nc.tensor.matmul(cps[h][:],
                                         vsb[kc][:, h * 65:h * 65 + 65],
                                         psbs[h][:], start=(kc == 0),
                                         stop=(kc == NK - 1))

                for kc in range(NK):
                    emit_s(kc)
                    if kc >= LOOKAHEAD:
                        emit_pv(kc - LOOKAHEAD)
                for kc in range(NK - LOOKAHEAD, NK):
                    emit_pv(kc)

                def normalize(hp=hp, cps=cps, heads=heads):
                    for h in heads:
                        rd = stat2.tile([1, 512], F32, tag="rd", name="rd")
                        nc.vector.reciprocal(rd[:], cps[h][64:65, :])
                        rdr = stat2.tile([1, 512], F32R, tag="rdr", name="rdr")
                        nc.vector.tensor_copy(rdr[:], rd[:])
                        bcp = pacc.tile([64, 512], F32, tag="mm", name=f"bcp{h}")
                        nc.tensor.matmul(bcp[:], ones[0:1, 0:64], rdr[:],
                                         start=True, stop=True)
                        bcs = tmp.tile([64, 512], F32, tag="sf", bufs=4,
                                       name="bcs")
                        nc.vector.tensor_copy(bcs[:], bcp[:])
                        if h % 2 == 0:
                            nc.vector.tensor_mul(packed[hp][0:64, :],
                                                 cps[h][0:64, :], bcs[:])
                        else:
                            ct = tmp.tile([64, 512], F32R, tag="sf", bufs=4,
                                          name="ct")
                            nc.vector.tensor_mul(ct[:], cps[h][0:64, :], bcs[:])
                            nc.sync.dma_start(packed[hp][64:128, :], ct[:])

                pending_norm.append(normalize)
                if len(pending_norm) > 1:
                    pending_norm.pop(0)()
            while pending_norm:
                pending_norm.pop(0)()
            s_attn.close()  # q/k/v/p freed

            # ---- O-proj + residual ----
            x2pool = ctx.enter_context(tc.tile_pool(name="x2pool", bufs=1))
            ops = [pacc.tile([128, 512], F32, tag="mm", name=f"ops{i}")
                   for i in range(NK)]
            for k in range(NK):
                wt = wpool.tile([128, E], F32R, tag="w", bufs=4, name="wok")
                nc.sync.dma_start(wt[:], d["wo"][k * 128:(k + 1) * 128, :])
                for m in range(NK):
                    nc.tensor.matmul(ops[m][:], wt[:, m * 128:(m + 1) * 128],
                                     packed[k][:], start=(k == 0),
                                     stop=(k == NK - 1))
            x2 = []
            x2r = []
            for m in range(NK):
                xot = tmp.tile([128, 512], F32, tag="sf", bufs=4, name="xot")
                nc.sync.dma_start(xot[:], d["xb"][m * 128:(m + 1) * 128, 0:512]
                                  .bitcast(F32))
                osb = tmp.tile([128, 512], F32, tag="sf", bufs=4, name="osb")
                nc.vector.tensor_scalar_add(osb[:], ops[m][:], bo[m][:])
                t = x2pool.tile([128, 512], F32, tag=f"x2{m}", name=f"x2_{m}")
                nc.vector.tensor_add(t[:], osb[:], xot[:])
                x2.append(t)
                tr = x2pool.tile([128, 512], F32R, tag=f"x2r{m}", name=f"x2r{m}")
                nc.scalar.copy(tr[:], t[:])
                x2r.append(tr)
        # xown, packed freed

        # ---- LN2 ----
        h2pool = ctx.enter_context(tc.tile_pool(name="h2pool", bufs=1))
        outp = ctx.enter_context(tc.tile_pool(name="outp", bufs=2))
        h2f = [h2pool.tile([128, 512], F32, tag=f"h2f{k}", name=f"h2f{k}")
               for k in range(NK)]
        layernorm(lambda k, sl: x2r[k][:, sl], lambda k, sl: x2[k][:, sl],
                  g2, b2, 512, lambda k, c: h2f[k][:])
        h2b = []
        for k in range(NK):
            t = h2pool.tile([128, 512], BF16, tag=f"h2b{k}", name=f"h2b{k}")
            if k % 2 == 0:
                nc.vector.tensor_copy(t[:], h2f[k][:])
            else:
                nc.scalar.copy(t[:], h2f[k][:])
            h2b.append(t)
            nc.sync.dma_start(d["h2out"][k * 128:(k + 1) * 128, :], t[:])

        # ---- router logits (full fp32) ----
        wgt = const.tile([128, NE * NK], F32, tag="wg")
        nc.sync.dma_start(wgt[:].rearrange("p (k e) -> p k e", e=NE),
                          d["wg"].rearrange("(k p) e -> p k e", p=128))
        gps = pacc.tile([NE, 512], F32, tag="mm", name="gps")
        for k in range(NK):
            nc.tensor.matmul(gps[:], wgt[:, k * NE:(k + 1) * NE], h2f[k][:],
                             start=(k == 0), stop=(k == NK - 1))
        lsb = outp.tile([NE, 512], F32, tag="l", name="lsb")
        nc.vector.tensor_copy(lsb[:], gps[:])
        nc.sync.dma_start(d["logits"][:], lsb[:])

        # ---- shared MLP (bf16) ----
        with tc.tile_pool(name="midpool", bufs=1) as midpool:
            mid = []
            for jg in range(4):
                mps = [pacc.tile([128, 512], F32, tag="mm", name=f"mps{jg}_{i}")
                       for i in range(8)]
                for k in range(NK):
                    wt = wpool.tile([128, 1024], BF16, tag="wb", bufs=8, name="sw1k")
                    nc.sync.dma_start(wt[:], d["sw1"][k * 128:(k + 1) * 128,
                                                      jg * 1024:(jg + 1) * 1024])
                    for j in range(8):
                        nc.tensor.matmul(mps[j][:], wt[:, j * 128:(j + 1) * 128],
                                         h2b[k][:], start=(k == 0),
                                         stop=(k == NK - 1))
                for j in range(8):
                    jj = jg * 8 + j
                    t = midpool.tile([128, 512], BF16, tag=f"mid{jj}",
                                     name=f"mid{jj}")
                    nc.scalar.activation(t[:], mps[j][:], AF.Gelu,
                                         bias=sb1t[jj][:])
                    mid.append(t)
            for mg in range(2):
                o2 = [pacc.tile([128, 512], F32, tag="mm", name=f"o2_{i}")
                      for i in range(4)]
                for j in range(DFF // 128):
                    wt = wpool.tile([128, 512], BF16, tag="wb", bufs=8,
                                    name="sw2j")
                    nc.sync.dma_start(wt[:], d["sw2"][j * 128:(j + 1) * 128,
                                                      mg * 512:(mg + 1) * 512])
                    for m in range(4):
                        nc.tensor.matmul(o2[m][:], wt[:, m * 128:(m + 1) * 128],
                                         mid[j][:], start=(j == 0),
                                         stop=(j == DFF // 128 - 1))
                for m in range(4):
                    mm2 = mg * 4 + m
                    sh = tmp.tile([128, 512], F32, tag="sf", bufs=4, name="sh")
                    nc.vector.tensor_scalar_add(sh[:], o2[m][:], sb2t[mm2][:])
                    pt = outp.tile([128, 512], F32, tag="pt", name="pt")
                    nc.vector.tensor_add(pt[:], sh[:], x2[mm2][:])
                    nc.sync.dma_start(d["partial"][mm2 * 128:(mm2 + 1) * 128, :],
                                      pt[:])

    nc.compile()
    return nc


def _build_launch_b(chunks):
    nc = bacc.Bacc("TRN2", target_bir_lowering=False, debug=False,
                   enable_asserts=True, num_devices=NCORES)
    C = sum(chunks)
    d = {}
    d["h2d"] = nc.dram_tensor("h2d", [E, C], BF16, kind="ExternalInput").ap()
    d["e1"] = nc.dram_tensor("e1", [E, DFF], BF16, kind="ExternalInput").ap()
    d["e2"] = nc.dram_tensor("e2", [DFF, E], BF16, kind="ExternalInput").ap()
    d["b1"] = nc.dram_tensor("b1", [DFF, 1], F32, kind="ExternalInput").ap()
    d["b2"] = nc.dram_tensor("b2", [E, 1], F32, kind="ExternalInput").ap()
    d["yout"] = nc.dram_tensor("yout", [E, C], F32, kind="ExternalOutput").ap()

    with tile.TileContext(nc) as tc, ExitStack() as ctx:
        pacc = ctx.enter_context(tc.tile_pool(name="pacc", bufs=8, space="PSUM"))
        const = ctx.enter_context(tc.tile_pool(name="const", bufs=1))
        hpool = ctx.enter_context(tc.tile_pool(name="hpool", bufs=1))
        wpool = ctx.enter_context(tc.tile_pool(name="wpool", bufs=14))
        midpool = ctx.enter_context(tc.tile_pool(name="midpool", bufs=1))
        ypool = ctx.enter_context(tc.tile_pool(name="ypool", bufs=2))

        b1w = const.tile([128, DFF // 128], F32, tag="b1w", name="b1w")
        nc.sync.dma_start(b1w[:].rearrange("p (k o) -> p k o", o=1),
                          d["b1"].rearrange("(k p) o -> p k o", p=128))
        b1t = [b1w[:, k:k + 1] for k in range(DFF // 128)]
        b2w = const.tile([128, NK], F32, tag="b2w", name="b2w")
        nc.sync.dma_start(b2w[:].rearrange("p (k o) -> p k o", o=1),
                          d["b2"].rearrange("(k p) o -> p k o", p=128))
        b2t = [b2w[:, k:k + 1] for k in range(NK)]
        h2d = [hpool.tile([128, C], BF16, tag=f"h{k}", name=f"h2d{k}")
               for k in range(NK)]

        off = 0
        for ci, cw in enumerate(chunks):
            csl = slice(off, off + cw)
            mid = []
            for jg in range(4):
                mps = [pacc.tile([128, cw], F32, tag="mm", name=f"bmps{i}")
                       for i in range(8)]
                for k in range(NK):
                    wt = wpool.tile([128, 1024], BF16, tag="w1", name="wt")
                    nc.sync.dma_start(wt[:], d["e1"][k * 128:(k + 1) * 128,
                                                     jg * 1024:(jg + 1) * 1024])
                    if ci == 0 and jg == 0:
                        nc.sync.dma_start(h2d[k][:],
                                          d["h2d"][k * 128:(k + 1) * 128, :])
                    for j in range(8):
                        nc.tensor.matmul(mps[j][:], wt[:, j * 128:(j + 1) * 128],
                                         h2d[k][:, csl], start=(k == 0),
                                         stop=(k == NK - 1))
                for j in range(8):
                    jj = jg * 8 + j
                    t = midpool.tile([128, cw], BF16, tag=f"mid{jj}_{ci % 2}",
                                     name=f"bmid{jj}")
                    nc.scalar.activation(t[:], mps[j][:], AF.Gelu,
                                         bias=b1t[jj][:])
                    mid.append(t)
            o2 = [pacc.tile([128, cw], F32, tag="mm", name=f"bo2_{i}")
                  for i in range(NK)]
            for j in range(DFF // 128):
                wt = wpool.tile([128, 1024], BF16, tag="w2", name="wt2")
                nc.sync.dma_start(wt[:], d["e2"][j * 128:(j + 1) * 128, :])
                for m in range(NK):
                    nc.tensor.matmul(o2[m][:], wt[:, m * 128:(m + 1) * 128],
                                     mid[j][:], start=(j == 0),
                                     stop=(j == DFF // 128 - 1))
            for m in range(NK):
                y = ypool.tile([128, cw], F32, tag="y", name="y")
                nc.scalar.activation(y[:], o2[m][:], AF.Identity,
                                     bias=b2t[m][:])
                nc.sync.dma_start(d["yout"][m * 128:(m + 1) * 128, csl], y[:])
            off += cw

    nc.compile()
    return nc


def _chunk_sizes(C):
    n = (C + 511) // 512
    base = C // n // 8 * 8
    sizes = [base] * n
    rem = C - base * n
    i = 0
    while rem > 0:
        step = min(8, rem)
        sizes[i] += step
        rem -= step
        i = (i + 1) % n
    assert sum(sizes) == C and all(s <= 512 for s in sizes)
    return sizes


def kernel(**inputs):
    global last_results, last_programs
    last_results = []
    last_programs = []

    f32 = lambda a: np.ascontiguousarray(np.asarray(a), dtype=np.float32)
    x = f32(inputs["x"])
    col = lambda a: f32(a).reshape(-1, 1)

    if "A" not in _cache:
        _cache["A"] = _build_launch_a()
    ncA = _cache["A"]

    wq, wk, wv, wo = (f32(inputs[k]) for k in ("wq", "wk", "wv", "wo"))
    sw1 = f32(inputs["sw1"]).astype(ml_dtypes.bfloat16)
    sw2 = f32(inputs["sw2"]).astype(ml_dtypes.bfloat16)
    onesd = np.ones((128, 128), np.float32)
    bvb = np.broadcast_to(f32(inputs["bv"]), (128, E)).copy()
    shared_in = dict(
        wq=wq, wk=wk, wv=wv, wo=wo,
        bq=col(inputs["bq"]), bk=col(inputs["bk"]), bo=col(inputs["bo"]),
        bvb=bvb,
        ln1_g=col(inputs["ln1_g"]), ln1_b=col(inputs["ln1_b"]),
        ln2_g=col(inputs["ln2_g"]), ln2_b=col(inputs["ln2_b"]),
        sw1=sw1, sb1=col(inputs["sb1"]), sw2=sw2, sb2=col(inputs["sb2"]),
        wg=f32(inputs["w_gate"]), onesd=onesd,
    )
    in_maps = []
    for c in range(NCORES):
        b, half = c // 2, c % 2
        xt = x[b].T  # [E, S]
        own = xt[:, half * 512:(half + 1) * 512]
        oth = xt[:, (1 - half) * 512:(2 - half) * 512]
        xb = np.ascontiguousarray(np.concatenate([own, oth], axis=1))
        in_maps.append({**shared_in, "xb": xb})

    resA = run_bass_kernel_spmd(ncA, in_maps, core_ids=list(range(NCORES)))
    last_results.append(("A", resA))
    last_programs.append(("A", ncA))

    partial = np.concatenate([resA.results[c]["partial"].T for c in range(NCORES)], 0)
    h2bf = np.concatenate([resA.results[c]["h2out"] for c in range(NCORES)], 1)
    logits = np.concatenate([resA.results[c]["logits"].T for c in range(NCORES)], 0)

    # ---- routing on host (mirrors reference, fp32) ----
    N = B * S
    order = np.argsort(-logits, axis=-1, kind="stable")
    top_idx = order[:, :TOPK]
    top_vals = np.take_along_axis(logits, top_idx, axis=-1)
    tv = top_vals - top_vals.max(-1, keepdims=True)
    te = np.exp(tv, dtype=np.float32)
    top_gates = te / te.sum(-1, keepdims=True)
    gates_dense = np.zeros((N, NE), np.float32)
    np.put_along_axis(gates_dense, top_idx, top_gates, axis=-1)
    lm = logits - logits.max(-1, keepdims=True)
    le = np.exp(lm, dtype=np.float32)
    probs = le / le.sum(-1, keepdims=True)
    P = probs.mean(0, dtype=np.float32)
    f = (gates_dense > 0).astype(np.float32).mean(0, dtype=np.float32)
    aux = np.float32(NE * np.sum(P * f, dtype=np.float32))

    # ---- dispatch ----
    sel_lists = [np.nonzero((top_idx == e).any(-1))[0] for e in range(NE)]
    counts = np.array([len(t) for t in sel_lists])
    C = int(max(512, -(-counts.max() // 8) * 8))
    chunks = tuple(_chunk_sizes(C))
    key = ("B", chunks)
    if key not in _cache:
        _cache[key] = _build_launch_b(list(chunks))
    ncB = _cache[key]

    in_maps_b = []
    ew1 = np.asarray(inputs["ew1"]).astype(ml_dtypes.bfloat16)
    ew2 = np.asarray(inputs["ew2"]).astype(ml_dtypes.bfloat16)
    eb1 = f32(inputs["eb1"]); eb2 = f32(inputs["eb2"])
    idxs = []
    for e in range(NE):
        idx = np.zeros(C, np.int64)
        idx[:counts[e]] = sel_lists[e]
        idxs.append(idx)
        h2d = np.ascontiguousarray(h2bf[:, idx])
        in_maps_b.append(dict(h2d=h2d, e1=np.ascontiguousarray(ew1[e]),
                              e2=np.ascontiguousarray(ew2[e]),
                              b1=eb1[e].reshape(-1, 1).astype(np.float32),
                              b2=eb2[e].reshape(-1, 1).astype(np.float32)))
    resB = run_bass_kernel_spmd(ncB, in_maps_b, core_ids=list(range(NCORES)))
    last_results.append(("B", resB))
    last_programs.append(("B", ncB))

    out = partial
    for e in range(NE):
        cnt = counts[e]
        if cnt == 0:
            continue
        y = resB.results[e]["yout"][:, :cnt].T  # [cnt, E]
        g = gates_dense[idxs[e][:cnt], e][:, None]
        out[idxs[e][:cnt]] += g * y
    return out.reshape(B, S, E).astype(np.float32), aux


# revision 45
# speedup vs baseline: 1.0575x; 1.0037x over previous
"""MoE transformer block on 8 TRN2 NeuronCores.

Launch A (data-parallel over tokens): per core = (batch b, seq half) -> 512
query tokens.  Attention path in fp32r (router-accuracy critical), shared MLP
in bf16.  Outputs partial = x2 + shared, h2 (bf16), router logits (fp32).

Host: top-2 routing, gates, aux loss, per-expert token dispatch.

Launch B (expert-parallel): core e runs expert e's MLP over its C dispatched
tokens, bf16 weights/activations.

Host: gated combine -> full output.
"""

import math
import numpy as np
import ml_dtypes
from contextlib import ExitStack

import concourse.bass as bass
import concourse.tile as tile
from concourse import bacc, mybir
from concourse.bass_utils import run_bass_kernel_spmd

F32 = mybir.dt.float32
F32R = mybir.dt.float32r
BF16 = mybir.dt.bfloat16
AF = mybir.ActivationFunctionType

B, S, E, H, NE, TOPK = 4, 1024, 1024, 16, 8, 2
DFF = 4 * E
DH = E // H
EPS = 1e-5
NCORES = 8
TOK = 512          # own tokens per core in launch A
NK = E // 128      # 8 feature tiles

_cache = {}
last_results = []   # [(name, BassKernelResults), ...] for test harness
last_programs = []  # [(name, Bacc), ...] for test harness timing


def _build_launch_a(trace=False):
    nc = bacc.Bacc("TRN2", target_bir_lowering=False, debug=False,
                   enable_asserts=True, num_devices=NCORES)
    d = {}
    def di(name, shape, dt):
        d[name] = nc.dram_tensor(name, shape, dt, kind="ExternalInput").ap()
    def do(name, shape, dt):
        d[name] = nc.dram_tensor(name, shape, dt, kind="ExternalOutput").ap()

    di("xb", [E, S], F32R)            # x[batch].T, own half first
    for w in ("wq", "wk", "wv", "wo"):
        di(w, [E, E], F32R)
    for b in ("bq", "bk", "bo"):
        di(b, [E, 1], F32)
    di("bvb", [128, E], F32)          # bv broadcast across partitions
    di("ln1_g", [E, 1], F32); di("ln1_b", [E, 1], F32)
    di("ln2_g", [E, 1], F32); di("ln2_b", [E, 1], F32)
    di("sw1", [E, DFF], BF16); di("sb1", [DFF, 1], F32)
    di("sw2", [DFF, E], BF16); di("sb2", [E, 1], F32)
    di("wg", [E, NE], F32)
    di("onesd", [128, 130], F32R)
    do("partial", [E, TOK], F32)      # x + attn + shared  (FM)
    do("h2out", [E, TOK], BF16)
    do("logits", [NE, TOK], F32)

    with tile.TileContext(nc) as tc, ExitStack() as ctx:
        pacc = ctx.enter_context(tc.tile_pool(name="pacc", bufs=8, space="PSUM"))
        const = ctx.enter_context(tc.tile_pool(name="const", bufs=1))
        wpool = ctx.enter_context(tc.tile_pool(name="wpool", bufs=3))
        stat = ctx.enter_context(tc.tile_pool(name="stat", bufs=2))
        tmp = ctx.enter_context(tc.tile_pool(name="tmp", bufs=2))

        ones = const.tile([128, 130], F32R, tag="ones")
        nc.sync.dma_start(ones[:], d["onesd"][:])

        def bias_tiles(name, n, tag):
            t = const.tile([128, n], F32, tag=tag, name=f"bt_{tag}")
            nc.sync.dma_start(t[:].rearrange("p (k o) -> p k o", o=1),
                              d[name].rearrange("(k p) o -> p k o", p=128))
            return [t[:, k:k + 1] for k in range(n)]

        g1 = bias_tiles("ln1_g", NK, "g1"); b1 = bias_tiles("ln1_b", NK, "b1")

        def layernorm(src_r, src_f, gt, bt, ncols, outs, chunk_sel=None):
            """src_r(k, sl)->fp32r AP (matmul rhs / Square in);
            src_f(k, sl)->fp32 AP; outs(k, c)->dest AP [128,512]."""
            nch = ncols // 512
            chunks = chunk_sel if chunk_sel is not None else range(nch)
            for c in chunks:
                sl = slice(c * 512, c * 512 + 512)
                # ones[:, 1] holds 1/E (exact 2^-10): psums are mean/meansq
                ms = pacc.tile([1, 512], F32, tag="mm", name="ms")
                qs = pacc.tile([1, 512], F32, tag="mm", name="qs")
                for k in range(NK):
                    sr = src_r(k, sl)
                    nc.tensor.matmul(ms[:], ones[:, 128:129], sr,
                                     start=(k == 0), stop=(k == NK - 1))
                    xsq = tmp.tile([128, 512], F32R, tag="xsq", name="xsq")
                    nc.scalar.activation(xsq[:], sr, AF.Square)
                    nc.tensor.matmul(qs[:], ones[:, 128:129], xsq[:],
                                     start=(k == 0), stop=(k == NK - 1))
                m2 = stat.tile([1, 512], F32, tag="scr", bufs=6, name="m2")
                nc.scalar.activation(m2[:], ms[:], AF.Square)
                var = stat.tile([1, 512], F32, tag="scr", bufs=6, name="var")
                nc.vector.tensor_sub(var[:], qs[:], m2[:])
                nc.vector.tensor_scalar_add(var[:], var[:], EPS)
                std = stat.tile([1, 512], F32, tag="scr", bufs=6, name="std")
                nc.scalar.activation(std[:], var[:], AF.Sqrt)
                rstd = stat.tile([1, 512], F32, tag="scr", bufs=6, name="rstd")
                nc.vector.reciprocal(rstd[:], std[:])
                m_r = stat.tile([1, 512], F32R, tag="scr", bufs=6, name="m_r")
                nc.vector.tensor_copy(m_r[:], ms[:])
                r_r = stat.tile([1, 512], F32R, tag="scr", bufs=6, name="r_r")
                nc.vector.tensor_copy(r_r[:], rstd[:])
                mb = pacc.tile([128, 512], F32, tag="mm", name="mb")
                nc.tensor.matmul(mb[:], ones[0:1, 0:128], m_r[:], start=True, stop=True)
                rb = pacc.tile([128, 512], F32, tag="mm", name="rb")
                nc.tensor.matmul(rb[:], ones[0:1, 0:128], r_r[:], start=True, stop=True)
                for k in range(NK):
                    t1 = tmp.tile([128, 512], F32, tag="t1", bufs=3, name="t1")
                    nc.vector.tensor_sub(t1[:], src_f(k, sl), mb[:])
                    nc.vector.tensor_mul(t1[:], t1[:], rb[:])
                    nc.vector.tensor_scalar(outs(k, c), t1[:], gt[k][:],
                                            bt[k][:], mybir.AluOpType.mult,
                                            mybir.AluOpType.add)

        cpool = ctx.enter_context(tc.tile_pool(name="cpool", bufs=1))
        if True:
            s_attn = ExitStack()
            qpool = s_attn.enter_context(tc.tile_pool(name="qpool", bufs=1))
            kpool = s_attn.enter_context(tc.tile_pool(name="kpool", bufs=1))
            vpool = s_attn.enter_context(tc.tile_pool(name="vpool", bufs=1))

            with tc.tile_pool(name="h1pool", bufs=1) as h1pool:
                h1 = [h1pool.tile([128, S], F32R, tag=f"h1{k}", name=f"h1{k}")
                      for k in range(NK)]

                with tc.tile_pool(name="xstrp", bufs=3) as xstrp:
                    def ln1_srcr(k, sl):
                        t = xstrp.tile([128, 512], F32R, tag="xstr",
                                       name="xstr")
                        nc.sync.dma_start(t[:],
                                          d["xb"][k * 128:(k + 1) * 128, sl])
                        return t[:]

                    def ln1_srcf(k, sl):
                        t = xstrp.tile([128, 512], F32R, tag="xstr",
                                       name="xstr2")
                        nc.sync.dma_start(t[:],
                                          d["xb"][k * 128:(k + 1) * 128, sl])
                        return t[:].bitcast(F32)

                    layernorm(ln1_srcr, ln1_srcf, g1, b1, S,
                              lambda k, c: h1[k][:, c * 512:(c + 1) * 512],
                              chunk_sel=[0])

                    g2 = bias_tiles("ln2_g", NK, "g2")
                    b2 = bias_tiles("ln2_b", NK, "b2")
                    bq = bias_tiles("bq", NK, "bq")
                    bk = bias_tiles("bk", NK, "bk")
                    bo = bias_tiles("bo", NK, "bo")
                    sb1t = bias_tiles("sb1", DFF // 128, "sb1")
                    sb2t = bias_tiles("sb2", NK, "sb2")
                    bvb_sb = const.tile([128, E], F32, tag="bvb")
                    nc.sync.dma_start(bvb_sb[:], d["bvb"][:])

                    # ---- Q (own 512 tokens) — overlaps LN1 chunk 1 ----
                    qps = [pacc.tile([128, 512], F32, tag="mm", name=f"qps{i}")
                           for i in range(NK)]
                    for k in range(NK):
                        wt = wpool.tile([128, E], F32R, tag="w", bufs=4,
                                        name="wqk")
                        nc.sync.dma_start(wt[:], d["wq"][k * 128:(k + 1) * 128, :])
                        for m in range(NK):
                            nc.tensor.matmul(qps[m][:],
                                             wt[:, m * 128:(m + 1) * 128],
                                             h1[k][:, 0:512], start=(k == 0),
                                             stop=(k == NK - 1))
                    qsb = []
                    for m in range(NK):
                        t = qpool.tile([128, 512], F32R, tag=f"q{m}",
                                       name=f"q{m}")
                        if m % 2 == 0:
                            nc.vector.tensor_scalar_add(t[:], qps[m][:], bq[m][:])
                        else:
                            nc.scalar.activation(t[:], qps[m][:], AF.Identity,
                                                 bias=bq[m][:])
                        qsb.append(t)

                    layernorm(ln1_srcr, ln1_srcf, g1, b1, S,
                              lambda k, c: h1[k][:, c * 512:(c + 1) * 512],
                              chunk_sel=[1])

                # ---- K (all 1024 tokens, two chunk passes) ----
                ksb = [kpool.tile([128, S], F32R, tag=f"k{m}", name=f"ksb{m}")
                       for m in range(NK)]
                for c2 in range(2):
                    kps = [pacc.tile([128, 512], F32, tag="mm", name=f"kps{i}")
                           for i in range(NK)]
                    for k in range(NK):
                        wt = wpool.tile([128, E], F32R, tag="w", bufs=4, name="wkk")
                        nc.sync.dma_start(wt[:], d["wk"][k * 128:(k + 1) * 128, :])
                        for m in range(NK):
                            nc.tensor.matmul(kps[m][:],
                                             wt[:, m * 128:(m + 1) * 128],
                                             h1[k][:, c2 * 512:(c2 + 1) * 512],
                                             start=(k == 0), stop=(k == NK - 1))
                    for m in range(NK):
                        if m % 2 == 0:
                            nc.vector.tensor_scalar_add(
                                ksb[m][:, c2 * 512:(c2 + 1) * 512],
                                kps[m][:], bk[m][:])
                        else:
                            nc.scalar.activation(
                                ksb[m][:, c2 * 512:(c2 + 1) * 512],
                                kps[m][:], AF.Identity, bias=bk[m][:])

                # ---- V (token-major, 65-strided per head, ones col) ----
                vsb = [vpool.tile([128, 16 * 65], F32R, tag=f"v{t}",
                                  name=f"vsb{t}") for t in range(NK)]
                for t in range(NK):
                    ov = vsb[t][:].rearrange("p (h e) -> p h e", e=65)[:, :, 64:65]
                    nc.scalar.copy(ov, ones[:, 0:16].rearrange(
                        "p (h e) -> p h e", e=1))
                for tg in range(2):
                    vps = {}
                    for t in range(4):
                        for f in range(2):
                            vps[(t, f)] = pacc.tile([128, 512], F32, tag="mm",
                                                    name=f"vps{t}_{f}")
                    for k in range(NK):
                        wt = wpool.tile([128, E], F32R, tag="w", bufs=4, name="wvk")
                        nc.sync.dma_start(wt[:], d["wv"][k * 128:(k + 1) * 128, :])
                        for t in range(4):
                            tt = tg * 4 + t
                            for f in range(2):
                                nc.tensor.matmul(
                                    vps[(t, f)][:],
                                    h1[k][:, tt * 128:tt * 128 + 128],
                                    wt[:, f * 512:(f + 1) * 512],
                                    start=(k == 0), stop=(k == NK - 1))
                    for t in range(4):
                        for f in range(2):
                            dst = vsb[tg * 4 + t][:, f * 520:(f + 1) * 520] \
                                .rearrange("p (h e) -> p h e", e=65)[:, :, 0:64]
                            src = vps[(t, f)][:].rearrange("p (h e) -> p h e", e=64)
                            nc.vector.tensor_add(
                                dst, src,
                                bvb_sb[:, f * 512:(f + 1) * 512].rearrange(
                                    "p (h e) -> p h e", e=64))
            # h1 freed here

            # ---- attention per head ----
            ppool = s_attn.enter_context(tc.tile_pool(name="ppool", bufs=6))
            stat2 = s_attn.enter_context(tc.tile_pool(name="stat2", bufs=2))
            packed = [cpool.tile([128, 512], F32R, tag=f"c{p}", name=f"packed{p}")
                      for p in range(NK)]
            LOOKAHEAD = 2
            pending_norm = []
            for hp in range(H // 2):
                heads = (2 * hp, 2 * hp + 1)
                cps = {}
                for h in heads:
                    cps[h] = pacc.tile([65, 512], F32, tag="mm", name=f"cps{h}")

                pend = {}  # kc -> {h: exp tile}

                def emit_s(kc):
                    psbs = {}
                    for h in heads:
                        ktile = ksb[hp][(h % 2) * 64:(h % 2) * 64 + 64, :]
                        qtile = qsb[hp][(h % 2) * 64:(h % 2) * 64 + 64, :]
                        sps = pacc.tile([128, 512], F32, tag="mm",
                                        name=f"sps{h}_{kc}")
                        nc.tensor.matmul(sps[:],
                                         ktile[:, kc * 128:kc * 128 + 128],
                                         qtile[:], start=True, stop=True)
                        psb = ppool.tile([128, 512], F32R, tag="p",
                                         name=f"p{h}_{kc}")
                        nc.scalar.activation(psb[:], sps[:], AF.Exp,
                                             scale=1.0 / math.sqrt(DH))
                        psbs[h] = psb
                    pend[kc] = psbs

                def emit_pv(kc):
                    psbs = pend.pop(kc)
                    for h in heads:
                        nc.tensor.matmul(cps[h][:],
                                         vsb[kc][:, h * 65:h * 65 + 65],
                                         psbs[h][:], start=(kc == 0),
                                         stop=(kc == NK - 1))

                for kc in range(NK):
                    emit_s(kc)
                    if kc >= LOOKAHEAD:
                        emit_pv(kc - LOOKAHEAD)
                for kc in range(NK - LOOKAHEAD, NK):
                    emit_pv(kc)

                def normalize(hp=hp, cps=cps, heads=heads):
                    for h in heads:
                        rd = stat2.tile([1, 512], F32, tag="rd", name="rd")
                        nc.vector.reciprocal(rd[:], cps[h][64:65, :])
                        rdr = stat2.tile([1, 512], F32R, tag="rdr", name="rdr")
                        nc.vector.tensor_copy(rdr[:], rd[:])
                        bcp = pacc.tile([64, 512], F32, tag="mm", name=f"bcp{h}")
                        nc.tensor.matmul(bcp[:], ones[0:1, 0:64], rdr[:],
                                         start=True, stop=True)
                        bcs = tmp.tile([64, 512], F32, tag="sf", bufs=4,
                                       name="bcs")
                        nc.vector.tensor_copy(bcs[:], bcp[:])
                        if h % 2 == 0:
                            nc.vector.tensor_mul(packed[hp][0:64, :],
                                                 cps[h][0:64, :], bcs[:])
                        else:
                            ct = tmp.tile([64, 512], F32R, tag="sf", bufs=4,
                                          name="ct")
                            nc.vector.tensor_mul(ct[:], cps[h][0:64, :], bcs[:])
                            nc.sync.dma_start(packed[hp][64:128, :], ct[:])

                pending_norm.append(normalize)
                if len(pending_norm) > 1:
                    pending_norm.pop(0)()
            while pending_norm:
                pending_norm.pop(0)()
            s_attn.close()  # q/k/v/p freed

            # ---- O-proj + residual ----
            x2pool = ctx.enter_context(tc.tile_pool(name="x2pool", bufs=1))
            ops = [pacc.tile([128, 512], F32, tag="mm", name=f"ops{i}")
                   for i in range(NK)]
            for k in range(NK):
                wt = wpool.tile([128, E], F32R, tag="w", bufs=4, name="wok")
                nc.sync.dma_start(wt[:], d["wo"][k * 128:(k + 1) * 128, :])
                for m in range(NK):
                    nc.tensor.matmul(ops[m][:], wt[:, m * 128:(m + 1) * 128],
                                     packed[k][:], start=(k == 0),
                                     stop=(k == NK - 1))
            x2 = []
            x2r = []
            for m in range(NK):
                xot = tmp.tile([128, 512], F32, tag="sf", bufs=4, name="xot")
                nc.sync.dma_start(xot[:], d["xb"][m * 128:(m + 1) * 128, 0:512]
                                  .bitcast(F32))
                osb = tmp.tile([128, 512], F32, tag="sf", bufs=4, name="osb")
                nc.vector.tensor_scalar_add(osb[:], ops[m][:], bo[m][:])
                t = x2pool.tile([128, 512], F32, tag=f"x2{m}", name=f"x2_{m}")
                nc.vector.tensor_add(t[:], osb[:], xot[:])
                x2.append(t)
                tr = x2pool.tile([128, 512], F32R, tag=f"x2r{m}", name=f"x2r{m}")
                nc.scalar.copy(tr[:], t[:])
                x2r.append(tr)
        # xown, packed freed

        # ---- LN2 ----
        h2pool = ctx.enter_context(tc.tile_pool(name="h2pool", bufs=1))
        outp = ctx.enter_context(tc.tile_pool(name="outp", bufs=2))
        h2f = [h2pool.tile([128, 512], F32, tag=f"h2f{k}", name=f"h2f{k}")
               for k in range(NK)]
        layernorm(lambda k, sl: x2r[k][:, sl], lambda k, sl: x2[k][:, sl],
                  g2, b2, 512, lambda k, c: h2f[k][:])
        h2b = []
        for k in range(NK):
            t = h2pool.tile([128, 512], BF16, tag=f"h2b{k}", name=f"h2b{k}")
            if k % 2 == 0:
                nc.vector.tensor_copy(t[:], h2f[k][:])
            else:
                nc.scalar.copy(t[:], h2f[k][:])
            h2b.append(t)
            nc.sync.dma_start(d["h2out"][k * 128:(k + 1) * 128, :], t[:])

        # ---- router logits (full fp32) ----
        wgt = const.tile([128, NE * NK], F32, tag="wg")
        nc.sync.dma_start(wgt[:].rearrange("p (k e) -> p k e", e=NE),
                          d["wg"].rearrange("(k p) e -> p k e", p=128))
        gps = pacc.tile([NE, 512], F32, tag="mm", name="gps")
        for k in range(NK):
            nc.tensor.matmul(gps[:], wgt[:, k * NE:(k + 1) * NE], h2f[k][:],
                             start=(k == 0), stop=(k == NK - 1))
        lsb = outp.tile([NE, 512], F32, tag="l", name="lsb")
        nc.vector.tensor_copy(lsb[:], gps[:])
        nc.sync.dma_start(d["logits"][:], lsb[:])

        # ---- shared MLP (bf16) ----
        with tc.tile_pool(name="midpool", bufs=1) as midpool:
            mid = []
            for jg in range(4):
                mps = [pacc.tile([128, 512], F32, tag="mm", name=f"mps{jg}_{i}")
                       for i in range(8)]
                for k in range(NK):
                    wt = wpool.tile([128, 1024], BF16, tag="wb", bufs=8, name="sw1k")
                    nc.sync.dma_start(wt[:], d["sw1"][k * 128:(k + 1) * 128,
                                                      jg * 1024:(jg + 1) * 1024])
                    for j in range(8):
                        nc.tensor.matmul(mps[j][:], wt[:, j * 128:(j + 1) * 128],
                                         h2b[k][:], start=(k == 0),
                                         stop=(k == NK - 1))
                for j in range(8):
                    jj = jg * 8 + j
                    t = midpool.tile([128, 512], BF16, tag=f"mid{jj}",
                                     name=f"mid{jj}")
                    nc.scalar.activation(t[:], mps[j][:], AF.Gelu,
                                         bias=sb1t[jj][:])
                    mid.append(t)
            for mg in range(2):
                o2 = [pacc.tile([128, 512], F32, tag="mm", name=f"o2_{i}")
                      for i in range(4)]
                for j in range(DFF // 128):
                    wt = wpool.tile([128, 512], BF16, tag="wb", bufs=8,
                                    name="sw2j")
                    nc.sync.dma_start(wt[:], d["sw2"][j * 128:(j + 1) * 128,
                                                      mg * 512:(mg + 1) * 512])
                    for m in range(4):
                        nc.tensor.matmul(o2[m][:], wt[:, m * 128:(m + 1) * 128],
                                         mid[j][:], start=(j == 0),
                                         stop=(j == DFF // 128 - 1))
                for m in range(4):
                    mm2 = mg * 4 + m
                    sh = tmp.tile([128, 512], F32, tag="sf", bufs=4, name="sh")
                    nc.vector.tensor_scalar_add(sh[:], o2[m][:], sb2t[mm2][:])
                    pt = outp.tile([128, 512], F32, tag="pt", name="pt")
                    nc.vector.tensor_add(pt[:], sh[:], x2[mm2][:])
                    nc.sync.dma_start(d["partial"][mm2 * 128:(mm2 + 1) * 128, :],
                                      pt[:])

    nc.compile()
    return nc


def _build_launch_b(chunks):
    nc = bacc.Bacc("TRN2", target_bir_lowering=False, debug=False,
                   enable_asserts=True, num_devices=NCORES)
    C = sum(chunks)
    d = {}
    d["h2d"] = nc.dram_tensor("h2d", [E, C], BF16, kind="ExternalInput").ap()
    d["e1"] = nc.dram_tensor("e1", [E, DFF], BF16, kind="ExternalInput").ap()
    d["e2"] = nc.dram_tensor("e2", [DFF, E], BF16, kind="ExternalInput").ap()
    d["b1"] = nc.dram_tensor("b1", [DFF, 1], F32, kind="ExternalInput").ap()
    d["b2"] = nc.dram_tensor("b2", [E, 1], F32, kind="ExternalInput").ap()
    d["yout"] = nc.dram_tensor("yout", [E, C], F32, kind="ExternalOutput").ap()

    with tile.TileContext(nc) as tc, ExitStack() as ctx:
        pacc = ctx.enter_context(tc.tile_pool(name="pacc", bufs=8, space="PSUM"))
        const = ctx.enter_context(tc.tile_pool(name="const", bufs=1))
        hpool = ctx.enter_context(tc.tile_pool(name="hpool", bufs=1))
        wpool = ctx.enter_context(tc.tile_pool(name="wpool", bufs=14))
        midpool = ctx.enter_context(tc.tile_pool(name="midpool", bufs=1))
        ypool = ctx.enter_context(tc.tile_pool(name="ypool", bufs=2))

        b1w = const.tile([128, DFF // 128], F32, tag="b1w", name="b1w")
        nc.sync.dma_start(b1w[:].rearrange("p (k o) -> p k o", o=1),
                          d["b1"].rearrange("(k p) o -> p k o", p=128))
        b1t = [b1w[:, k:k + 1] for k in range(DFF // 128)]
        b2w = const.tile([128, NK], F32, tag="b2w", name="b2w")
        nc.sync.dma_start(b2w[:].rearrange("p (k o) -> p k o", o=1),
                          d["b2"].rearrange("(k p) o -> p k o", p=128))
        b2t = [b2w[:, k:k + 1] for k in range(NK)]
        h2d = [hpool.tile([128, C], BF16, tag=f"h{k}", name=f"h2d{k}")
               for k in range(NK)]

        off = 0
        for ci, cw in enumerate(chunks):
            csl = slice(off, off + cw)
            mid = []
            for jg in range(4):
                mps = [pacc.tile([128, cw], F32, tag="mm", name=f"bmps{i}")
                       for i in range(8)]
                for k in range(NK):
                    wt = wpool.tile([128, 1024], BF16, tag="w1", name="wt")
                    nc.sync.dma_start(wt[:], d["e1"][k * 128:(k + 1) * 128,
                                                     jg * 1024:(jg + 1) * 1024])
                    if ci == 0 and jg == 0:
                        nc.sync.dma_start(h2d[k][:],
                                          d["h2d"][k * 128:(k + 1) * 128, :])
                    for j in range(8):
                        nc.tensor.matmul(mps[j][:], wt[:, j * 128:(j + 1) * 128],
                                         h2d[k][:, csl], start=(k == 0),
                                         stop=(k == NK - 1))
                for j in range(8):
                    jj = jg * 8 + j
                    t = midpool.tile([128, cw], BF16, tag=f"mid{jj}_{ci % 2}",
                                     name=f"bmid{jj}")
                    nc.scalar.activation(t[:], mps[j][:], AF.Gelu,
                                         bias=b1t[jj][:])
                    mid.append(t)
            o2 = [pacc.tile([128, cw], F32, tag="mm", name=f"bo2_{i}")
                  for i in range(NK)]
            for j in range(DFF // 128):
                wt = wpool.tile([128, 1024], BF16, tag="w2", name="wt2")
                nc.sync.dma_start(wt[:], d["e2"][j * 128:(j + 1) * 128, :])
                for m in range(NK):
                    nc.tensor.matmul(o2[m][:], wt[:, m * 128:(m + 1) * 128],
                                     mid[j][:], start=(j == 0),
                                     stop=(j == DFF // 128 - 1))
            for m in range(NK):
                y = ypool.tile([128, cw], F32, tag="y", name="y")
                nc.scalar.activation(y[:], o2[m][:], AF.Identity,
                                     bias=b2t[m][:])
                nc.sync.dma_start(d["yout"][m * 128:(m + 1) * 128, csl], y[:])
            off += cw

    nc.compile()
    return nc


def _chunk_sizes(C):
    n = (C + 511) // 512
    base = C // n // 8 * 8
    sizes = [base] * n
    rem = C - base * n
    i = 0
    while rem > 0:
        step = min(8, rem)
        sizes[i] += step
        rem -= step
        i = (i + 1) % n
    assert sum(sizes) == C and all(s <= 512 for s in sizes)
    return sizes


def kernel(**inputs):
    global last_results, last_programs
    last_results = []
    last_programs = []

    f32 = lambda a: np.ascontiguousarray(np.asarray(a), dtype=np.float32)
    x = f32(inputs["x"])
    col = lambda a: f32(a).reshape(-1, 1)

    if "A" not in _cache:
        _cache["A"] = _build_launch_a()
    ncA = _cache["A"]

    wq, wk, wv, wo = (f32(inputs[k]) for k in ("wq", "wk", "wv", "wo"))
    sw1 = f32(inputs["sw1"]).astype(ml_dtypes.bfloat16)
    sw2 = f32(inputs["sw2"]).astype(ml_dtypes.bfloat16)
    onesd = np.ones((128, 130), np.float32)
    onesd[:, 128] = 1.0 / E
    bvb = np.broadcast_to(f32(inputs["bv"]), (128, E)).copy()
    shared_in = dict(
        wq=wq, wk=wk, wv=wv, wo=wo,
        bq=col(inputs["bq"]), bk=col(inputs["bk"]), bo=col(inputs["bo"]),
        bvb=bvb,
        ln1_g=col(inputs["ln1_g"]), ln1_b=col(inputs["ln1_b"]),
        ln2_g=col(inputs["ln2_g"]), ln2_b=col(inputs["ln2_b"]),
        sw1=sw1, sb1=col(inputs["sb1"]), sw2=sw2, sb2=col(inputs["sb2"]),
        wg=f32(inputs["w_gate"]), onesd=onesd,
    )
    in_maps = []
    for c in range(NCORES):
        b, half = c // 2, c % 2
        xt = x[b].T  # [E, S]
        own = xt[:, half * 512:(half + 1) * 512]
        oth = xt[:, (1 - half) * 512:(2 - half) * 512]
        xb = np.ascontiguousarray(np.concatenate([own, oth], axis=1))
        in_maps.append({**shared_in, "xb": xb})

    resA = run_bass_kernel_spmd(ncA, in_maps, core_ids=list(range(NCORES)))
    last_results.append(("A", resA))
    last_programs.append(("A", ncA))

    partial = np.concatenate([resA.results[c]["partial"].T for c in range(NCORES)], 0)
    h2bf = np.concatenate([resA.results[c]["h2out"] for c in range(NCORES)], 1)
    logits = np.concatenate([resA.results[c]["logits"].T for c in range(NCORES)], 0)

    # ---- routing on host (mirrors reference, fp32) ----
    N = B * S
    order = np.argsort(-logits, axis=-1, kind="stable")
    top_idx = order[:, :TOPK]
    top_vals = np.take_along_axis(logits, top_idx, axis=-1)
    tv = top_vals - top_vals.max(-1, keepdims=True)
    te = np.exp(tv, dtype=np.float32)
    top_gates = te / te.sum(-1, keepdims=True)
    gates_dense = np.zeros((N, NE), np.float32)
    np.put_along_axis(gates_dense, top_idx, top_gates, axis=-1)
    lm = logits - logits.max(-1, keepdims=True)
    le = np.exp(lm, dtype=np.float32)
    probs = le / le.sum(-1, keepdims=True)
    P = probs.mean(0, dtype=np.float32)
    f = (gates_dense > 0).astype(np.float32).mean(0, dtype=np.float32)
    aux = np.float32(NE * np.sum(P * f, dtype=np.float32))

    # ---- dispatch ----
    sel_lists = [np.nonzero((top_idx == e).any(-1))[0] for e in range(NE)]
    counts = np.array([len(t) for t in sel_lists])
    C = int(max(512, -(-counts.max() // 8) * 8))
    chunks = tuple(_chunk_sizes(C))
    key = ("B", chunks)
    if key not in _cache:
        _cache[key] = _build_launch_b(list(chunks))
    ncB = _cache[key]

    in_maps_b = []
    ew1 = np.asarray(inputs["ew1"]).astype(ml_dtypes.bfloat16)
    ew2 = np.asarray(inputs["ew2"]).astype(ml_dtypes.bfloat16)
    eb1 = f32(inputs["eb1"]); eb2 = f32(inputs["eb2"])
    idxs = []
    for e in range(NE):
        idx = np.zeros(C, np.int64)
        idx[:counts[e]] = sel_lists[e]
        idxs.append(idx)
        h2d = np.ascontiguousarray(h2bf[:, idx])
        in_maps_b.append(dict(h2d=h2d, e1=np.ascontiguousarray(ew1[e]),
                              e2=np.ascontiguousarray(ew2[e]),
                              b1=eb1[e].reshape(-1, 1).astype(np.float32),
                              b2=eb2[e].reshape(-1, 1).astype(np.float32)))
    resB = run_bass_kernel_spmd(ncB, in_maps_b, core_ids=list(range(NCORES)))
    last_results.append(("B", resB))
    last_programs.append(("B", ncB))

    out = partial
    for e in range(NE):
        cnt = counts[e]
        if cnt == 0:
            continue
        y = resB.results[e]["yout"][:, :cnt].T  # [cnt, E]
        g = gates_dense[idxs[e][:cnt], e][:, None]
        out[idxs[e][:cnt]] += g * y
    return out.reshape(B, S, E).astype(np.float32), aux


# revision 46
# speedup vs baseline: 1.0691x; 1.0110x over previous
"""MoE transformer block on 8 TRN2 NeuronCores.

Launch A (data-parallel over tokens): per core = (batch b, seq half) -> 512
query tokens.  Attention path in fp32r (router-accuracy critical), shared MLP
in bf16.  Outputs partial = x2 + shared, h2 (bf16), router logits (fp32).

Host: top-2 routing, gates, aux loss, per-expert token dispatch.

Launch B (expert-parallel): core e runs expert e's MLP over its C dispatched
tokens, bf16 weights/activations.

Host: gated combine -> full output.
"""

import math
import numpy as np
import ml_dtypes
from contextlib import ExitStack

import concourse.bass as bass
import concourse.tile as tile
from concourse import bacc, mybir
from concourse.bass_utils import run_bass_kernel_spmd

F32 = mybir.dt.float32
F32R = mybir.dt.float32r
BF16 = mybir.dt.bfloat16
AF = mybir.ActivationFunctionType

B, S, E, H, NE, TOPK = 4, 1024, 1024, 16, 8, 2
DFF = 4 * E
DH = E // H
EPS = 1e-5
NCORES = 8
TOK = 512          # own tokens per core in launch A
NK = E // 128      # 8 feature tiles

_cache = {}
last_results = []   # [(name, BassKernelResults), ...] for test harness
last_programs = []  # [(name, Bacc), ...] for test harness timing


def _build_launch_a(trace=False):
    nc = bacc.Bacc("TRN2", target_bir_lowering=False, debug=False,
                   enable_asserts=True, num_devices=NCORES)
    d = {}
    def di(name, shape, dt):
        d[name] = nc.dram_tensor(name, shape, dt, kind="ExternalInput").ap()
    def do(name, shape, dt):
        d[name] = nc.dram_tensor(name, shape, dt, kind="ExternalOutput").ap()

    di("xb", [E, S], F32R)            # x[batch].T, own half first
    for w in ("wq", "wk", "wv", "wo"):
        di(w, [E, E], F32R)
    for b in ("bq", "bk", "bo"):
        di(b, [E, 1], F32)
    di("bvb", [128, E], F32)          # bv broadcast across partitions
    di("ln1_g", [E, 1], F32); di("ln1_b", [E, 1], F32)
    di("ln2_g", [E, 1], F32); di("ln2_b", [E, 1], F32)
    di("sw1", [E, DFF], BF16); di("sb1", [DFF, 1], F32)
    di("sw2", [DFF, E], BF16); di("sb2", [E, 1], F32)
    di("wg", [E, NE], F32)
    di("onesd", [128, 130], F32R)
    do("partial", [E, TOK], F32)      # x + attn + shared  (FM)
    do("h2out", [E, TOK], BF16)
    do("logits", [NE, TOK], F32)

    with tile.TileContext(nc) as tc, ExitStack() as ctx:
        pacc = ctx.enter_context(tc.tile_pool(name="pacc", bufs=8, space="PSUM"))
        const = ctx.enter_context(tc.tile_pool(name="const", bufs=1))
        wpool = ctx.enter_context(tc.tile_pool(name="wpool", bufs=3))
        stat = ctx.enter_context(tc.tile_pool(name="stat", bufs=2))
        tmp = ctx.enter_context(tc.tile_pool(name="tmp", bufs=2))

        ones = const.tile([128, 130], F32R, tag="ones")
        nc.sync.dma_start(ones[:], d["onesd"][:])

        def bias_tiles(name, n, tag):
            t = const.tile([128, n], F32, tag=tag, name=f"bt_{tag}")
            nc.sync.dma_start(t[:].rearrange("p (k o) -> p k o", o=1),
                              d[name].rearrange("(k p) o -> p k o", p=128))
            return [t[:, k:k + 1] for k in range(n)]

        g1 = bias_tiles("ln1_g", NK, "g1"); b1 = bias_tiles("ln1_b", NK, "b1")

        def layernorm(src_r, src_f, gt, bt, ncols, outs, chunk_sel=None):
            """src_r(k, sl)->fp32r AP (matmul rhs / Square in);
            src_f(k, sl)->fp32 AP; outs(k, c)->dest AP [128,512]."""
            nch = ncols // 512
            chunks = chunk_sel if chunk_sel is not None else range(nch)
            for c in chunks:
                sl = slice(c * 512, c * 512 + 512)
                # ones[:, 1] holds 1/E (exact 2^-10): psums are mean/meansq
                ms = pacc.tile([1, 512], F32, tag="mm", name="ms")
                qs = pacc.tile([1, 512], F32, tag="mm", name="qs")
                for k in range(NK):
                    sr = src_r(k, sl)
                    nc.tensor.matmul(ms[:], ones[:, 128:129], sr,
                                     start=(k == 0), stop=(k == NK - 1))
                    xsq = tmp.tile([128, 512], F32R, tag="xsq", name="xsq")
                    nc.scalar.activation(xsq[:], sr, AF.Square)
                    nc.tensor.matmul(qs[:], ones[:, 128:129], xsq[:],
                                     start=(k == 0), stop=(k == NK - 1))
                m2 = stat.tile([1, 512], F32, tag="scr", bufs=6, name="m2")
                nc.scalar.activation(m2[:], ms[:], AF.Square)
                var = stat.tile([1, 512], F32, tag="scr", bufs=6, name="var")
                nc.vector.tensor_sub(var[:], qs[:], m2[:])
                nc.vector.tensor_scalar_add(var[:], var[:], EPS)
                std = stat.tile([1, 512], F32, tag="scr", bufs=6, name="std")
                nc.scalar.activation(std[:], var[:], AF.Sqrt)
                rstd = stat.tile([1, 512], F32, tag="scr", bufs=6, name="rstd")
                nc.vector.reciprocal(rstd[:], std[:])
                m_r = stat.tile([1, 512], F32R, tag="scr", bufs=6, name="m_r")
                nc.vector.tensor_copy(m_r[:], ms[:])
                r_r = stat.tile([1, 512], F32R, tag="scr", bufs=6, name="r_r")
                nc.vector.tensor_copy(r_r[:], rstd[:])
                mb = pacc.tile([128, 512], F32, tag="mm", name="mb")
                nc.tensor.matmul(mb[:], ones[0:1, 0:128], m_r[:], start=True, stop=True)
                rb = pacc.tile([128, 512], F32, tag="mm", name="rb")
                nc.tensor.matmul(rb[:], ones[0:1, 0:128], r_r[:], start=True, stop=True)
                for k in range(NK):
                    t1 = tmp.tile([128, 512], F32, tag="t1", bufs=3, name="t1")
                    nc.vector.tensor_sub(t1[:], src_f(k, sl), mb[:])
                    nc.vector.tensor_mul(t1[:], t1[:], rb[:])
                    nc.vector.tensor_scalar(outs(k, c), t1[:], gt[k][:],
                                            bt[k][:], mybir.AluOpType.mult,
                                            mybir.AluOpType.add)

        cpool = ctx.enter_context(tc.tile_pool(name="cpool", bufs=1))
        if True:
            s_attn = ExitStack()
            qpool = s_attn.enter_context(tc.tile_pool(name="qpool", bufs=1))
            kpool = s_attn.enter_context(tc.tile_pool(name="kpool", bufs=1))
            vpool = s_attn.enter_context(tc.tile_pool(name="vpool", bufs=1))

            with tc.tile_pool(name="h1pool", bufs=1) as h1pool:
                h1 = [h1pool.tile([128, S], F32R, tag=f"h1{k}", name=f"h1{k}")
                      for k in range(NK)]

                with tc.tile_pool(name="xstrp", bufs=4) as xstrp:
                    def ln1_srcr(k, sl):
                        t = xstrp.tile([128, 512], F32R, tag="xstr",
                                       name="xstr")
                        nc.sync.dma_start(t[:],
                                          d["xb"][k * 128:(k + 1) * 128, sl])
                        return t[:]

                    def ln1_srcf(k, sl):
                        t = xstrp.tile([128, 512], F32R, tag="xstr",
                                       name="xstr2")
                        nc.sync.dma_start(t[:],
                                          d["xb"][k * 128:(k + 1) * 128, sl])
                        return t[:].bitcast(F32)

                    layernorm(ln1_srcr, ln1_srcf, g1, b1, S,
                              lambda k, c: h1[k][:, c * 512:(c + 1) * 512],
                              chunk_sel=[0])

                    g2 = bias_tiles("ln2_g", NK, "g2")
                    b2 = bias_tiles("ln2_b", NK, "b2")
                    bq = bias_tiles("bq", NK, "bq")
                    bk = bias_tiles("bk", NK, "bk")
                    bo = bias_tiles("bo", NK, "bo")
                    sb1t = bias_tiles("sb1", DFF // 128, "sb1")
                    sb2t = bias_tiles("sb2", NK, "sb2")
                    bvb_sb = const.tile([128, E], F32, tag="bvb")
                    nc.sync.dma_start(bvb_sb[:], d["bvb"][:])

                    # ---- Q (own 512 tokens) — overlaps LN1 chunk 1 ----
                    qps = [pacc.tile([128, 512], F32, tag="mm", name=f"qps{i}")
                           for i in range(NK)]
                    for k in range(NK):
                        wt = wpool.tile([128, E], F32R, tag="w", bufs=5,
                                        name="wqk")
                        nc.sync.dma_start(wt[:], d["wq"][k * 128:(k + 1) * 128, :])
                        for m in range(NK):
                            nc.tensor.matmul(qps[m][:],
                                             wt[:, m * 128:(m + 1) * 128],
                                             h1[k][:, 0:512], start=(k == 0),
                                             stop=(k == NK - 1))
                    qsb = []
                    for m in range(NK):
                        t = qpool.tile([128, 512], F32R, tag=f"q{m}",
                                       name=f"q{m}")
                        if m % 2 == 0:
                            nc.vector.tensor_scalar_add(t[:], qps[m][:], bq[m][:])
                        else:
                            nc.scalar.activation(t[:], qps[m][:], AF.Identity,
                                                 bias=bq[m][:])
                        qsb.append(t)

                    layernorm(ln1_srcr, ln1_srcf, g1, b1, S,
                              lambda k, c: h1[k][:, c * 512:(c + 1) * 512],
                              chunk_sel=[1])

                # ---- K (all 1024 tokens, two chunk passes) ----
                ksb = [kpool.tile([128, S], F32R, tag=f"k{m}", name=f"ksb{m}")
                       for m in range(NK)]
                for c2 in range(2):
                    kps = [pacc.tile([128, 512], F32, tag="mm", name=f"kps{i}")
                           for i in range(NK)]
                    for k in range(NK):
                        wt = wpool.tile([128, E], F32R, tag="w", bufs=5, name="wkk")
                        nc.sync.dma_start(wt[:], d["wk"][k * 128:(k + 1) * 128, :])
                        for m in range(NK):
                            nc.tensor.matmul(kps[m][:],
                                             wt[:, m * 128:(m + 1) * 128],
                                             h1[k][:, c2 * 512:(c2 + 1) * 512],
                                             start=(k == 0), stop=(k == NK - 1))
                    for m in range(NK):
                        if m % 2 == 0:
                            nc.vector.tensor_scalar_add(
                                ksb[m][:, c2 * 512:(c2 + 1) * 512],
                                kps[m][:], bk[m][:])
                        else:
                            nc.scalar.activation(
                                ksb[m][:, c2 * 512:(c2 + 1) * 512],
                                kps[m][:], AF.Identity, bias=bk[m][:])

                # ---- V (token-major, 65-strided per head, ones col) ----
                vsb = [vpool.tile([128, 16 * 65], F32R, tag=f"v{t}",
                                  name=f"vsb{t}") for t in range(NK)]
                for t in range(NK):
                    ov = vsb[t][:].rearrange("p (h e) -> p h e", e=65)[:, :, 64:65]
                    nc.scalar.copy(ov, ones[:, 0:16].rearrange(
                        "p (h e) -> p h e", e=1))
                for tg in range(2):
                    vps = {}
                    for t in range(4):
                        for f in range(2):
                            vps[(t, f)] = pacc.tile([128, 512], F32, tag="mm",
                                                    name=f"vps{t}_{f}")
                    for k in range(NK):
                        wt = wpool.tile([128, E], F32R, tag="w", bufs=5, name="wvk")
                        nc.sync.dma_start(wt[:], d["wv"][k * 128:(k + 1) * 128, :])
                        for t in range(4):
                            tt = tg * 4 + t
                            for f in range(2):
                                nc.tensor.matmul(
                                    vps[(t, f)][:],
                                    h1[k][:, tt * 128:tt * 128 + 128],
                                    wt[:, f * 512:(f + 1) * 512],
                                    start=(k == 0), stop=(k == NK - 1))
                    for t in range(4):
                        for f in range(2):
                            dst = vsb[tg * 4 + t][:, f * 520:(f + 1) * 520] \
                                .rearrange("p (h e) -> p h e", e=65)[:, :, 0:64]
                            src = vps[(t, f)][:].rearrange("p (h e) -> p h e", e=64)
                            nc.vector.tensor_add(
                                dst, src,
                                bvb_sb[:, f * 512:(f + 1) * 512].rearrange(
                                    "p (h e) -> p h e", e=64))
            # h1 freed here

            # ---- attention per head ----
            ppool = s_attn.enter_context(tc.tile_pool(name="ppool", bufs=6))
            stat2 = s_attn.enter_context(tc.tile_pool(name="stat2", bufs=2))
            packed = [cpool.tile([128, 512], F32R, tag=f"c{p}", name=f"packed{p}")
                      for p in range(NK)]
            LOOKAHEAD = 2
            pending_norm = []
            for hp in range(H // 2):
                heads = (2 * hp, 2 * hp + 1)
                cps = {}
                for h in heads:
                    cps[h] = pacc.tile([65, 512], F32, tag="mm", name=f"cps{h}")

                pend = {}  # kc -> {h: exp tile}

                def emit_s(kc):
                    psbs = {}
                    for h in heads:
                        ktile = ksb[hp][(h % 2) * 64:(h % 2) * 64 + 64, :]
                        qtile = qsb[hp][(h % 2) * 64:(h % 2) * 64 + 64, :]
                        sps = pacc.tile([128, 512], F32, tag="mm",
                                        name=f"sps{h}_{kc}")
                        nc.tensor.matmul(sps[:],
                                         ktile[:, kc * 128:kc * 128 + 128],
                                         qtile[:], start=True, stop=True)
                        psb = ppool.tile([128, 512], F32R, tag="p",
                                         name=f"p{h}_{kc}")
                        nc.scalar.activation(psb[:], sps[:], AF.Exp,
                                             scale=1.0 / math.sqrt(DH))
                        psbs[h] = psb
                    pend[kc] = psbs

                def emit_pv(kc):
                    psbs = pend.pop(kc)
                    for h in heads:
                        nc.tensor.matmul(cps[h][:],
                                         vsb[kc][:, h * 65:h * 65 + 65],
                                         psbs[h][:], start=(kc == 0),
                                         stop=(kc == NK - 1))

                for kc in range(NK):
                    emit_s(kc)
                    if kc >= LOOKAHEAD:
                        emit_pv(kc - LOOKAHEAD)
                for kc in range(NK - LOOKAHEAD, NK):
                    emit_pv(kc)

                def normalize(hp=hp, cps=cps, heads=heads):
                    for h in heads:
                        rd = stat2.tile([1, 512], F32, tag="rd", name="rd")
                        nc.vector.reciprocal(rd[:], cps[h][64:65, :])
                        rdr = stat2.tile([1, 512], F32R, tag="rdr", name="rdr")
                        nc.vector.tensor_copy(rdr[:], rd[:])
                        bcp = pacc.tile([64, 512], F32, tag="mm", name=f"bcp{h}")
                        nc.tensor.matmul(bcp[:], ones[0:1, 0:64], rdr[:],
                                         start=True, stop=True)
                        bcs = tmp.tile([64, 512], F32, tag="sf", bufs=4,
                                       name="bcs")
                        nc.vector.tensor_copy(bcs[:], bcp[:])
                        if h % 2 == 0:
                            nc.vector.tensor_mul(packed[hp][0:64, :],
                                                 cps[h][0:64, :], bcs[:])
                        else:
                            ct = tmp.tile([64, 512], F32R, tag="sf", bufs=4,
                                          name="ct")
                            nc.vector.tensor_mul(ct[:], cps[h][0:64, :], bcs[:])
                            nc.sync.dma_start(packed[hp][64:128, :], ct[:])

                pending_norm.append(normalize)
                if len(pending_norm) > 1:
                    pending_norm.pop(0)()
            while pending_norm:
                pending_norm.pop(0)()
            s_attn.close()  # q/k/v/p freed

            # ---- O-proj + residual ----
            x2pool = ctx.enter_context(tc.tile_pool(name="x2pool", bufs=1))
            ops = [pacc.tile([128, 512], F32, tag="mm", name=f"ops{i}")
                   for i in range(NK)]
            for k in range(NK):
                wt = wpool.tile([128, E], F32R, tag="w", bufs=5, name="wok")
                nc.sync.dma_start(wt[:], d["wo"][k * 128:(k + 1) * 128, :])
                for m in range(NK):
                    nc.tensor.matmul(ops[m][:], wt[:, m * 128:(m + 1) * 128],
                                     packed[k][:], start=(k == 0),
                                     stop=(k == NK - 1))
            x2 = []
            x2r = []
            for m in range(NK):
                xot = tmp.tile([128, 512], F32, tag="sf", bufs=4, name="xot")
                nc.sync.dma_start(xot[:], d["xb"][m * 128:(m + 1) * 128, 0:512]
                                  .bitcast(F32))
                osb = tmp.tile([128, 512], F32, tag="sf", bufs=4, name="osb")
                nc.vector.tensor_scalar_add(osb[:], ops[m][:], bo[m][:])
                t = x2pool.tile([128, 512], F32, tag=f"x2{m}", name=f"x2_{m}")
                nc.vector.tensor_add(t[:], osb[:], xot[:])
                x2.append(t)
                tr = x2pool.tile([128, 512], F32R, tag=f"x2r{m}", name=f"x2r{m}")
                nc.scalar.copy(tr[:], t[:])
                x2r.append(tr)
        # xown, packed freed

        # ---- LN2 ----
        h2pool = ctx.enter_context(tc.tile_pool(name="h2pool", bufs=1))
        outp = ctx.enter_context(tc.tile_pool(name="outp", bufs=2))
        h2f = [h2pool.tile([128, 512], F32, tag=f"h2f{k}", name=f"h2f{k}")
               for k in range(NK)]
        layernorm(lambda k, sl: x2r[k][:, sl], lambda k, sl: x2[k][:, sl],
                  g2, b2, 512, lambda k, c: h2f[k][:])
        h2b = []
        for k in range(NK):
            t = h2pool.tile([128, 512], BF16, tag=f"h2b{k}", name=f"h2b{k}")
            if k % 2 == 0:
                nc.vector.tensor_copy(t[:], h2f[k][:])
            else:
                nc.scalar.copy(t[:], h2f[k][:])
            h2b.append(t)
            nc.sync.dma_start(d["h2out"][k * 128:(k + 1) * 128, :], t[:])

        # ---- router logits (full fp32) ----
        wgt = const.tile([128, NE * NK], F32, tag="wg")
        nc.sync.dma_start(wgt[:].rearrange("p (k e) -> p k e", e=NE),
                          d["wg"].rearrange("(k p) e -> p k e", p=128))
        gps = pacc.tile([NE, 512], F32, tag="mm", name="gps")
        for k in range(NK):
            nc.tensor.matmul(gps[:], wgt[:, k * NE:(k + 1) * NE], h2f[k][:],
                             start=(k == 0), stop=(k == NK - 1))
        lsb = outp.tile([NE, 512], F32, tag="l", name="lsb")
        nc.vector.tensor_copy(lsb[:], gps[:])
        nc.sync.dma_start(d["logits"][:], lsb[:])

        # ---- shared MLP (bf16) ----
        with tc.tile_pool(name="midpool", bufs=1) as midpool:
            mid = []
            for jg in range(4):
                mps = [pacc.tile([128, 512], F32, tag="mm", name=f"mps{jg}_{i}")
                       for i in range(8)]
                for k in range(NK):
                    wt = wpool.tile([128, 1024], BF16, tag="wb", bufs=8, name="sw1k")
                    nc.sync.dma_start(wt[:], d["sw1"][k * 128:(k + 1) * 128,
                                                      jg * 1024:(jg + 1) * 1024])
                    for j in range(8):
                        nc.tensor.matmul(mps[j][:], wt[:, j * 128:(j + 1) * 128],
                                         h2b[k][:], start=(k == 0),
                                         stop=(k == NK - 1))
                for j in range(8):
                    jj = jg * 8 + j
                    t = midpool.tile([128, 512], BF16, tag=f"mid{jj}",
                                     name=f"mid{jj}")
                    nc.scalar.activation(t[:], mps[j][:], AF.Gelu,
                                         bias=sb1t[jj][:])
                    mid.append(t)
            for mg in range(2):
                o2 = [pacc.tile([128, 512], F32, tag="mm", name=f"o2_{i}")
                      for i in range(4)]
                for j in range(DFF // 128):
                    wt = wpool.tile([128, 512], BF16, tag="wb", bufs=8,
                                    name="sw2j")
                    nc.sync.dma_start(wt[:], d["sw2"][j * 128:(j + 1) * 128,
                                                      mg * 512:(mg + 1) * 512])
                    for m in range(4):
                        nc.tensor.matmul(o2[m][:], wt[:, m * 128:(m + 1) * 128],
                                         mid[j][:], start=(j == 0),
                                         stop=(j == DFF // 128 - 1))
                for m in range(4):
                    mm2 = mg * 4 + m
                    sh = tmp.tile([128, 512], F32, tag="sf", bufs=4, name="sh")
                    nc.vector.tensor_scalar_add(sh[:], o2[m][:], sb2t[mm2][:])
                    pt = outp.tile([128, 512], F32, tag="pt", name="pt")
                    nc.vector.tensor_add(pt[:], sh[:], x2[mm2][:])
                    nc.sync.dma_start(d["partial"][mm2 * 128:(mm2 + 1) * 128, :],
                                      pt[:])

    nc.compile()
    return nc


def _build_launch_b(chunks):
    nc = bacc.Bacc("TRN2", target_bir_lowering=False, debug=False,
                   enable_asserts=True, num_devices=NCORES)
    C = sum(chunks)
    d = {}
    d["h2d"] = nc.dram_tensor("h2d", [E, C], BF16, kind="ExternalInput").ap()
    d["e1"] = nc.dram_tensor("e1", [E, DFF], BF16, kind="ExternalInput").ap()
    d["e2"] = nc.dram_tensor("e2", [DFF, E], BF16, kind="ExternalInput").ap()
    d["b1"] = nc.dram_tensor("b1", [DFF, 1], F32, kind="ExternalInput").ap()
    d["b2"] = nc.dram_tensor("b2", [E, 1], F32, kind="ExternalInput").ap()
    d["yout"] = nc.dram_tensor("yout", [E, C], F32, kind="ExternalOutput").ap()

    with tile.TileContext(nc) as tc, ExitStack() as ctx:
        pacc = ctx.enter_context(tc.tile_pool(name="pacc", bufs=8, space="PSUM"))
        const = ctx.enter_context(tc.tile_pool(name="const", bufs=1))
        hpool = ctx.enter_context(tc.tile_pool(name="hpool", bufs=1))
        wpool = ctx.enter_context(tc.tile_pool(name="wpool", bufs=14))
        midpool = ctx.enter_context(tc.tile_pool(name="midpool", bufs=1))
        ypool = ctx.enter_context(tc.tile_pool(name="ypool", bufs=2))

        b1w = const.tile([128, DFF // 128], F32, tag="b1w", name="b1w")
        nc.sync.dma_start(b1w[:].rearrange("p (k o) -> p k o", o=1),
                          d["b1"].rearrange("(k p) o -> p k o", p=128))
        b1t = [b1w[:, k:k + 1] for k in range(DFF // 128)]
        b2w = const.tile([128, NK], F32, tag="b2w", name="b2w")
        nc.sync.dma_start(b2w[:].rearrange("p (k o) -> p k o", o=1),
                          d["b2"].rearrange("(k p) o -> p k o", p=128))
        b2t = [b2w[:, k:k + 1] for k in range(NK)]
        h2d = [hpool.tile([128, C], BF16, tag=f"h{k}", name=f"h2d{k}")
               for k in range(NK)]

        off = 0
        for ci, cw in enumerate(chunks):
            csl = slice(off, off + cw)
            mid = []
            for jg in range(4):
                mps = [pacc.tile([128, cw], F32, tag="mm", name=f"bmps{i}")
                       for i in range(8)]
                for k in range(NK):
                    wt = wpool.tile([128, 1024], BF16, tag="w1", name="wt")
                    nc.sync.dma_start(wt[:], d["e1"][k * 128:(k + 1) * 128,
                                                     jg * 1024:(jg + 1) * 1024])
                    if ci == 0 and jg == 0:
                        nc.sync.dma_start(h2d[k][:],
                                          d["h2d"][k * 128:(k + 1) * 128, :])
                    for j in range(8):
                        nc.tensor.matmul(mps[j][:], wt[:, j * 128:(j + 1) * 128],
                                         h2d[k][:, csl], start=(k == 0),
                                         stop=(k == NK - 1))
                for j in range(8):
                    jj = jg * 8 + j
                    t = midpool.tile([128, cw], BF16, tag=f"mid{jj}_{ci % 2}",
                                     name=f"bmid{jj}")
                    nc.scalar.activation(t[:], mps[j][:], AF.Gelu,
                                         bias=b1t[jj][:])
                    mid.append(t)
            o2 = [pacc.tile([128, cw], F32, tag="mm", name=f"bo2_{i}")
                  for i in range(NK)]
            for j in range(DFF // 128):
                wt = wpool.tile([128, 1024], BF16, tag="w2", name="wt2")
                nc.sync.dma_start(wt[:], d["e2"][j * 128:(j + 1) * 128, :])
                for m in range(NK):
                    nc.tensor.matmul(o2[m][:], wt[:, m * 128:(m + 1) * 128],
                                     mid[j][:], start=(j == 0),
                                     stop=(j == DFF // 128 - 1))
            for m in range(NK):
                y = ypool.tile([128, cw], F32, tag="y", name="y")
                nc.scalar.activation(y[:], o2[m][:], AF.Identity,
                                     bias=b2t[m][:])
                nc.sync.dma_start(d["yout"][m * 128:(m + 1) * 128, csl], y[:])
            off += cw

    nc.compile()
    return nc


def _chunk_sizes(C):
    n = (C + 511) // 512
    base = C // n // 8 * 8
    sizes = [base] * n
    rem = C - base * n
    i = 0
    while rem > 0:
        step = min(8, rem)
        sizes[i] += step
        rem -= step
        i = (i + 1) % n
    assert sum(sizes) == C and all(s <= 512 for s in sizes)
    return sizes


def kernel(**inputs):
    global last_results, last_programs
    last_results = []
    last_programs = []

    f32 = lambda a: np.ascontiguousarray(np.asarray(a), dtype=np.float32)
    x = f32(inputs["x"])
    col = lambda a: f32(a).reshape(-1, 1)

    if "A" not in _cache:
        _cache["A"] = _build_launch_a()
    ncA = _cache["A"]

    wq, wk, wv, wo = (f32(inputs[k]) for k in ("wq", "wk", "wv", "wo"))
    sw1 = f32(inputs["sw1"]).astype(ml_dtypes.bfloat16)
    sw2 = f32(inputs["sw2"]).astype(ml_dtypes.bfloat16)
    onesd = np.ones((128, 130), np.float32)
    onesd[:, 128] = 1.0 / E
    bvb = np.broadcast_to(f32(inputs["bv"]), (128, E)).copy()
    shared_in = dict(
        wq=wq, wk=wk, wv=wv, wo=wo,
        bq=col(inputs["bq"]), bk=col(inputs["bk"]), bo=col(inputs["bo"]),
        bvb=bvb,
        ln1_g=col(inputs["ln1_g"]), ln1_b=col(inputs["ln1_b"]),
        ln2_g=col(inputs["ln2_g"]), ln2_b=col(inputs["ln2_b"]),
        sw1=sw1, sb1=col(inputs["sb1"]), sw2=sw2, sb2=col(inputs["sb2"]),
        wg=f32(inputs["w_gate"]), onesd=onesd,
    )
    in_maps = []
    for c in range(NCORES):
        b, half = c // 2, c % 2
        xt = x[b].T  # [E, S]
        own = xt[:, half * 512:(half + 1) * 512]
        oth = xt[:, (1 - half) * 512:(2 - half) * 512]
        xb = np.ascontiguousarray(np.concatenate([own, oth], axis=1))
        in_maps.append({**shared_in, "xb": xb})

    resA = run_bass_kernel_spmd(ncA, in_maps, core_ids=list(range(NCORES)))
    last_results.append(("A", resA))
    last_programs.append(("A", ncA))

    partial = np.concatenate([resA.results[c]["partial"].T for c in range(NCORES)], 0)
    h2bf = np.concatenate([resA.results[c]["h2out"] for c in range(NCORES)], 1)
    logits = np.concatenate([resA.results[c]["logits"].T for c in range(NCORES)], 0)

    # ---- routing on host (mirrors reference, fp32) ----
    N = B * S
    order = np.argsort(-logits, axis=-1, kind="stable")
    top_idx = order[:, :TOPK]
    top_vals = np.take_along_axis(logits, top_idx, axis=-1)
    tv = top_vals - top_vals.max(-1, keepdims=True)
    te = np.exp(tv, dtype=np.float32)
    top_gates = te / te.sum(-1, keepdims=True)
    gates_dense = np.zeros((N, NE), np.float32)
    np.put_along_axis(gates_dense, top_idx, top_gates, axis=-1)
    lm = logits - logits.max(-1, keepdims=True)
    le = np.exp(lm, dtype=np.float32)
    probs = le / le.sum(-1, keepdims=True)
    P = probs.mean(0, dtype=np.float32)
    f = (gates_dense > 0).astype(np.float32).mean(0, dtype=np.float32)
    aux = np.float32(NE * np.sum(P * f, dtype=np.float32))

    # ---- dispatch ----
    sel_lists = [np.nonzero((top_idx == e).any(-1))[0] for e in range(NE)]
    counts = np.array([len(t) for t in sel_lists])
    C = int(max(512, -(-counts.max() // 8) * 8))
    chunks = tuple(_chunk_sizes(C))
    key = ("B", chunks)
    if key not in _cache:
        _cache[key] = _build_launch_b(list(chunks))
    ncB = _cache[key]

    in_maps_b = []
    ew1 = np.asarray(inputs["ew1"]).astype(ml_dtypes.bfloat16)
    ew2 = np.asarray(inputs["ew2"]).astype(ml_dtypes.bfloat16)
    eb1 = f32(inputs["eb1"]); eb2 = f32(inputs["eb2"])
    idxs = []
    for e in range(NE):
        idx = np.zeros(C, np.int64)
        idx[:counts[e]] = sel_lists[e]
        idxs.append(idx)
        h2d = np.ascontiguousarray(h2bf[:, idx])
        in_maps_b.append(dict(h2d=h2d, e1=np.ascontiguousarray(ew1[e]),
                              e2=np.ascontiguousarray(ew2[e]),
                              b1=eb1[e].reshape(-1, 1).astype(np.float32),
                              b2=eb2[e].reshape(-1, 1).astype(np.float32)))
    resB = run_bass_kernel_spmd(ncB, in_maps_b, core_ids=list(range(NCORES)))
    last_results.append(("B", resB))
    last_programs.append(("B", ncB))

    out = partial
    for e in range(NE):
        cnt = counts[e]
        if cnt == 0:
            continue
        y = resB.results[e]["yout"][:, :cnt].T  # [cnt, E]
        g = gates_dense[idxs[e][:cnt], e][:, None]
        out[idxs[e][:cnt]] += g * y
    return out.reshape(B, S, E).astype(np.float32), aux
